# revision 33
# baseline (speedup 1.0000x reference)
"""AttnGCN layer on 8 TRN2 NeuronCores — data-parallel over batch.

Per-core (one sample b):
  q = x @ Wq + bq ; k = x @ Wk + bk            (fp8 PE matmuls)
  sT[i,o] = k_i . q_o  + C'*eT[i,o]            (scores transposed; mask folded
                                                into PSUM via lhsT=e-block
                                                matmuls against a scaled
                                                identity — transposes e free)
  pT = exp(alpha*sT - C)                        (ACT, masked entries -> ~0)
  S[o] = sum_i pT[i,o]                          (ones-vector PE matmul)
  ctxT[e,o] = sum_i x8[i,e] * pT[i,o]           (PE, accumulated over i-blocks)
  P[o,c] = ctxT@Wc + S*x  ... LN(P) == LN(x + softmax@x@Wc) exactly
                                                (LayerNorm is scale-invariant
                                                 per row -> 1/S never needed)
  out = LN(P)*gamma + beta                      (DVE bn_stats/bn_aggr epilogue)

Self-contained: hardcodes shapes from the problem spec.
"""

import math
from contextlib import ExitStack

import numpy as np

import concourse.bass as bass
import concourse.tile as tile
from concourse import mybir
from concourse.vector_clock import ScopedClock

F32 = mybir.dt.float32
F32R = mybir.dt.float32r
BF16 = mybir.dt.bfloat16
FP8 = mybir.dt.float8e4

B = 8
N = 2048
D = 512
P = 128
NB = N // P       # 16 i-blocks
EC = D // P       # 4 chunks of the embed/dff dim
OC = N // 512     # 4 o-chunks of 512 attn rows
ALPHA = 1.0 / math.sqrt(D)
CPRIME = 128.0            # mask scale inside PSUM (fp8e4 max finite is 240)
SHIFT = 3.0               # softmax-invariant shift keeping exp() in fp8 range
CBIAS = CPRIME * ALPHA + SHIFT  # subtracted in the exp bias


# ---------------------------------------------------------------------------
# Workaround: walrus CoreV3 rejects >2 sem waits on the TileContext final
# drain ("Too many sync wait commands"). Hoist waits onto preceding nops.
def _patched_drain_and_barrier(self, tick_clock, wait_clock):
    nc = self.nc
    carrier = nc.sync.nop(nofuse=True)
    wait_clock.add_sem_waits(carrier.ins, ScopedClock({None: tick_clock.global_clock}))
    si = carrier.ins.sync_info
    waits = list(si.on_wait) if si and si.on_wait else []
    if len(waits) > 1:
        si.on_wait = waits[:1]
        for w in waits[1:]:
            n2 = nc.sync.nop(nofuse=True)
            n2.ins.sync_info = mybir.SyncInfo(on_wait=[w], on_update=[])
    nc.sync.drain()
    nc.all_engine_barrier()
    assert self.sems is not None
    popped = nc._tile_sem_poison_stack.pop()
    assert popped is self._sem_poison
    nc.clear_and_free_semaphores(list(self.sems.allocated().values()))
    nc.all_engine_barrier()


def _apply_patches():
    tile.TileContext._drain_and_barrier = _patched_drain_and_barrier


def _split_excess_waits(nc, limit=1):
    """walrus CoreV2/V3 codegen rejects instructions with >2 sem waits;
    hoist excess waits onto same-engine no-ops inserted just before."""
    n = 0
    for fn in nc.m.functions:
        for blk in fn.blocks:
            out = []
            changed = False
            for inst in blk.instructions:
                si = inst.sync_info
                waits = list(si.on_wait) if si and si.on_wait else []
                if len(waits) > limit:
                    keep = waits[-limit:]
                    for w in waits[:-limit]:
                        n += 1
                        nop = mybir.InstNoOp(name=f"I-wsplit-{n}", ins=[], outs=[])
                        nop.engine = inst.engine
                        nop.sync_info = mybir.SyncInfo(on_wait=[w], on_update=[])
                        out.append(nop)
                    si.on_wait = keep
                    changed = True
                out.append(inst)
            if changed:
                blk.instructions = out
    return n


def _identity(nc, ap, diag):
    nc.gpsimd.memset(ap, 0.0)
    nc.gpsimd.affine_select(
        out=ap,
        in_=ap,
        compare_op=mybir.AluOpType.not_equal,
        fill=diag,
        base=0,
        pattern=[[-1, ap.shape[0]]],
        channel_multiplier=1,
    )


def build_nc():
    nc = bass.Bass()
    x_ext = nc.declare_dram_parameter("node_fts", [N, D], F32, isOutput=False)
    e_ext = nc.declare_dram_parameter("rel_edges", [N, N], F32, isOutput=False)
    wq_ext = nc.declare_dram_parameter("Wq", [D, D], F32, isOutput=False)
    bq_ext = nc.declare_dram_parameter("bq", [D], F32, isOutput=False)
    wk_ext = nc.declare_dram_parameter("Wk", [D, D], F32, isOutput=False)
    bk_ext = nc.declare_dram_parameter("bk", [D], F32, isOutput=False)
    wc_ext = nc.declare_dram_parameter("Wc", [D, D], F32, isOutput=False)
    g_ext = nc.declare_dram_parameter("gamma", [D], F32, isOutput=False)
    be_ext = nc.declare_dram_parameter("beta", [D], F32, isOutput=False)
    out_ext = nc.declare_dram_parameter("out", [N, D], BF16, isOutput=True)

    with tile.TileContext(nc) as tc, ExitStack() as ctx:
        singles = ctx.enter_context(tc.tile_pool(name="singles", bufs=1))
        wstage = ctx.enter_context(tc.tile_pool(name="wstage", bufs=2))
        xtp = ctx.enter_context(tc.tile_pool(name="xtp", bufs=2))
        ptp = ctx.enter_context(tc.tile_pool(name="ptp", bufs=4))
        ctxp = ctx.enter_context(tc.tile_pool(name="ctxp", bufs=2))
        rowp = ctx.enter_context(tc.tile_pool(name="rowp", bufs=2))
        epi = ctx.enter_context(tc.tile_pool(name="epi", bufs=2))
        xlnp = ctx.enter_context(tc.tile_pool(name="xlnp", bufs=5))
        sps = ctx.enter_context(tc.tile_pool(name="sps", bufs=3, space="PSUM"))
        ctxps_pool = ctx.enter_context(tc.tile_pool(name="ctxps", bufs=1, space="PSUM"))
        spsum = ctx.enter_context(tc.tile_pool(name="spsum", bufs=1, space="PSUM"))

        # ---- persistent tiles -------------------------------------------
        xs_tiles = [
            singles.tile([P, 4, D], F32, tag=f"xs{g}", name=f"xs{g}")
            for g in range(4)
        ]
        x8g = [
            singles.tile([P, 4, D], FP8, tag=f"x8g{g}", name=f"x8g{g}")
            for g in range(4)
        ]
        # 16 e-quarters, fp8, each [128 (o in block), 4 (o blocks), 512 (i)]
        e8_all = [
            singles.tile([P, 4, 512], FP8, tag=f"e8_{j}", name=f"e8_{j}")
            for j in range(16)
        ]
        qt8 = singles.tile([P, EC, N], FP8, tag="qt8")
        kt8 = singles.tile([P, EC, N], FP8, tag="kt8")
        wq8 = singles.tile([P, EC, D], FP8, tag="wq8")
        wk8 = singles.tile([P, EC, D], FP8, tag="wk8")
        wc8 = singles.tile([P, EC, D], FP8, tag="wc8")
        bq_row = singles.tile([1, D], F32, tag="bq_row")
        bk_row = singles.tile([1, D], F32, tag="bk_row")
        g_row = singles.tile([1, D], F32, tag="g_row")
        be_row = singles.tile([1, D], F32, tag="be_row")
        bqkt = singles.tile([P, 2 * EC], F32, tag="bqkt")
        gamma_b16 = singles.tile([P, D], BF16, tag="gamma_b16")
        beta_b16 = singles.tile([P, D], BF16, tag="beta_b16")
        ident32 = singles.tile([P, P], F32, tag="ident32")
        maskid8 = singles.tile([P, P], FP8, tag="maskid8")
        ones8 = singles.tile([P, 2, 16], FP8, tag="ones8")
        one32 = singles.tile([1, 1], F32, tag="one32")
        eps_t = singles.tile([P, 1], F32, tag="eps_t")
        cbias_t = singles.tile([P, 1], F32, tag="cbias_t")

        # gpsimd queue: identity build only, then DMA triggers ASAP;
        # the rest of the constants go on DVE so the SWDGE loads start early
        _identity(nc, ident32, 1.0)
        nc.vector.tensor_scalar(
            out=maskid8, in0=ident32, scalar1=CPRIME, scalar2=None,
            op0=mybir.AluOpType.mult,
        )
        nc.vector.memset(ones8, 1.0)
        nc.vector.memset(one32, 1.0)
        nc.vector.memset(eps_t, 1e-5)
        nc.vector.memset(cbias_t, -CBIAS)

        # ---- input loads, split across the three DMA paths --------------
        # sync+scalar HWDGE: x (f32, prep-critical; the ~1us trigger cost
        # per DMA serializes on one engine); SWDGE (casting): weights->fp8
        # then edges->fp8. Non-critical loads get tile_wait_until so the
        # priority scheduler doesn't float them into the x stream's window
        # (dep-free DMA triggers otherwise all issue at t~0 and the SDMA
        # round-robin starves x: xs3 was landing at 29-33us).
        def load_xs(g):
            eng = nc.sync if g % 2 == 0 else nc.scalar
            eng.dma_start(
                out=xs_tiles[g],
                in_=x_ext[g * 4 * P : (g + 1) * 4 * P, :].rearrange(
                    "(ib p) e -> p ib e", p=P
                ),
            )

        def load_e(j):  # j = oc*4 + q
            oc, q = j // 4, j % 4
            nc.gpsimd.dma_start(
                out=e8_all[j],
                in_=e_ext[
                    oc * 512 : (oc + 1) * 512, q * 512 : (q + 1) * 512
                ].rearrange("(s p) f -> p s f", p=P),
            )

        def load_w(w_ext, w8):
            nc.gpsimd.dma_start(
                out=w8, in_=w_ext[:, :].rearrange("(ec p) f -> p ec f", p=P)
            )

        # prep-critical loads only; e-quarters for oc>0 are issued inside the
        # main loop (one oc ahead) so they don't steal DMA bandwidth from x/w
        for g in range(4):
            load_xs(g)
        load_w(wq_ext, wq8)
        load_w(wk_ext, wk8)
        for j, ms in enumerate((0.008, 0.011, 0.015, 0.019)):
            with tc.tile_wait_until(ms):
                load_e(j)
        with tc.tile_wait_until(0.023):
            load_w(wc_ext, wc8)

        # ---- HAM warmup: dummy matmul burst while the first DMAs land ----
        # PE clock-gate needs ~3.4us of sustained activity to go 1.2->2.4GHz;
        # burn the DMA-wait with throwaway matmuls so prep runs warm. The
        # operand is an uninitialized tile (no writer -> no deps) so the
        # burst starts right after the NEFF preamble, before the constants.
        warm_src = singles.tile([P, P], FP8, tag="warm_src")
        nc.vector.memset(warm_src, 1.0)
        warm_ps = sps.tile([P, 512], F32, tag="sps")
        for j in range(72):
            nc.tensor.matmul(
                out=warm_ps[:, (j % 4) * P : (j % 4 + 1) * P],
                lhsT=warm_src,
                rhs=warm_src,
                start=True,
                stop=True,
                skip_group_check=True,
            )

        # vector params as single-row loads (1 descriptor each; the old
        # [128,D] broadcast DMAs generated 4-byte descriptors and cost
        # 8.3us of engine time per trigger). Broadcast/transpose on-chip.
        def _row(ap_1d):
            return bass.AP(
                tensor=ap_1d.tensor, offset=ap_1d.offset, ap=[[0, 1], *ap_1d.ap]
            )

        with tc.tile_wait_until(0.006):
            nc.scalar.dma_start(out=bq_row, in_=_row(bq_ext[:]))
            nc.scalar.dma_start(out=bk_row, in_=_row(bk_ext[:]))
            nc.scalar.dma_start(out=g_row, in_=_row(g_ext[:]))
            nc.scalar.dma_start(out=be_row, in_=_row(be_ext[:]))

        # broadcast rows across partitions via a K=1 PE matmul
        # (ones_col.T @ row), then one cast each to bf16
        ones_col = singles.tile([1, P], F32, tag="ones_col")
        nc.vector.memset(ones_col, 1.0)
        for row, dst in ((g_row, gamma_b16), (be_row, beta_b16)):
            bc_ps = sps.tile([P, 512], F32, tag="sps")
            nc.tensor.matmul(
                out=bc_ps,
                lhsT=ones_col,
                rhs=row,
                start=True,
                stop=True,
                skip_group_check=True,
            )
            nc.vector.tensor_copy(out=dst, in_=bc_ps)

        # biases per-partition [p, fc] (f = fc*128 + p) via tiny PE
        # transposes, like the S column later
        bqk_ps = sps.tile([P, 2 * EC], F32, tag="sps")
        for j in range(8):
            src = bq_row if j < 4 else bk_row
            nc.tensor.matmul(
                out=bqk_ps[:, j : j + 1],
                lhsT=src[0:1, (j % 4) * P : (j % 4 + 1) * P],
                rhs=one32,
                is_transpose=True,
                start=(j == 0),
                stop=(j == 7),
                skip_group_check=True,
            )
        nc.vector.tensor_copy(out=bqkt, in_=bqk_ps)
        bqt = bqkt[:, 0:EC]
        bkt = bqkt[:, EC : 2 * EC]

        # ---- per-group prep: x8 cast, transposes, q/k projections -------
        def prep_group(g):
            xs = xs_tiles[g]
            nc.vector.tensor_copy(out=x8g[g], in_=xs)
            xt = xtp.tile([P, EC, 512], FP8, tag="xt")
            for ec in range(EC):
                tp = sps.tile([P, 512], F32, tag="sps")
                for k4 in range(4):
                    nc.tensor.transpose(
                        out=tp[:, k4 * P : (k4 + 1) * P],
                        in_=xs[:, k4, ec * P : (ec + 1) * P],
                        identity=ident32,
                    )
                nc.vector.tensor_copy(out=xt[:, ec, :], in_=tp)
            for w8, bt, dst in ((wq8, bqt, qt8), (wk8, bkt, kt8)):
                for fc in range(EC):
                    ps = sps.tile([P, 512], F32, tag="sps")
                    for dc in (0, 2):
                        nc.tensor.matmul(
                            out=ps,
                            lhsT=w8[:, dc : dc + 2, fc * P : (fc + 1) * P],
                            rhs=xt[:, dc : dc + 2, :],
                            start=(dc == 0),
                            stop=(dc == 2),
                            perf_mode=mybir.MatmulPerfMode.DoubleRow,
                            skip_group_check=True,
                        )
                    # bias + fp8 cast: q on ACT, k on DVE — splitting the
                    # PSUM->SBUF drain keeps either engine off the prep
                    # critical path (ACT alone was the prep bottleneck)
                    if dst is qt8:
                        nc.scalar.activation(
                            out=dst[:, fc, g * 512 : (g + 1) * 512],
                            in_=ps,
                            func=mybir.ActivationFunctionType.Identity,
                            bias=bt[:, fc : fc + 1],
                            scale=1.0,
                        )
                    else:
                        nc.vector.tensor_scalar(
                            out=dst[:, fc, g * 512 : (g + 1) * 512],
                            in0=ps,
                            scalar1=bt[:, fc : fc + 1],
                            scalar2=None,
                            op0=mybir.AluOpType.add,
                        )

        # ---- main loop over o-chunks ------------------------------------
        # The per-oc epilogue is software-pipelined into the NEXT oc's
        # i-block loop: the PE engine queue is in-order, so emitting
        # epilogue PE work (s_col transposes, Wc matmuls) contiguously at
        # the oc boundary head-of-line blocks ready score matmuls behind a
        # DVE dependency chain. Interleaving stages at fixed i-blocks keeps
        # PE dense and spreads the DVE/gpsimd chain across the whole oc.
        class Epi:
            def __init__(self, oc, ctx_ps, s_ps):
                self.oc = oc
                self.ctx_ps = ctx_ps
                self.s_ps = s_ps
                self.xln = []

            def head(self):
                # free the S row bank + ctx_ps banks ASAP; split the copies
                # across ACT and DVE so neither queue's backlog delays the
                # next oc's first ctx matmul
                self.s_sb = rowp.tile([1, 512], F32, tag="s_sb")
                nc.scalar.copy(out=self.s_sb, in_=self.s_ps)
                self.ctx8 = ctxp.tile([P, EC, 512], FP8, tag="ctx8")
                for ec in range(EC):
                    eng_copy = (
                        nc.vector.tensor_copy if ec < 2 else nc.scalar.copy
                    )
                    eng_copy(out=self.ctx8[:, ec, :], in_=self.ctx_ps[:, ec, :])

            def scol(self):
                # S per-partition via tiny PE transposes (no reciprocal:
                # LayerNorm is scale-invariant per row, so we feed it
                # P = ctx_unnorm@Wc + S*x instead of x + (ctx_unnorm/S)@Wc)
                s_col = sps.tile([P, 4], F32, tag="sps")
                for j in range(4):
                    nc.tensor.matmul(
                        out=s_col[:, j : j + 1],
                        lhsT=self.s_sb[0:1, j * P : (j + 1) * P],
                        rhs=one32,
                        is_transpose=True,
                        start=(j == 0),
                        stop=(j == 3),
                        skip_group_check=True,
                    )
                self.s_colv = rowp.tile([P, 4], F32, tag="s_colv")
                nc.vector.tensor_copy(out=self.s_colv, in_=s_col)
                self.xsum4 = epi.tile([P, 4], F32, tag="xsum4")
                self.sq4 = epi.tile([P, 4], F32, tag="sq4")

            def wc(self, t):
                opre = sps.tile([P, 512], F32, tag="sps")
                for dc in (0, 2):
                    nc.tensor.matmul(
                        out=opre,
                        lhsT=self.ctx8[:, dc : dc + 2, t * P : (t + 1) * P],
                        rhs=wc8[:, dc : dc + 2, :],
                        start=(dc == 0),
                        stop=(dc == 2),
                        perf_mode=mybir.MatmulPerfMode.DoubleRow,
                        skip_group_check=True,
                    )
                # x_ln = S*xs + ctx@Wc, with sum(x_ln) accumulated for free;
                # the second moment runs on gpsimd so the DVE never does
                # bn_stats (keeps the DVE queue short at oc boundaries)
                x_ln = xlnp.tile([P, D], F32, tag="x_ln")
                nc.vector.scalar_tensor_tensor(
                    out=x_ln,
                    in0=xs_tiles[self.oc][:, t, :],
                    scalar=self.s_colv[:, t : t + 1],
                    in1=opre,
                    op0=mybir.AluOpType.mult,
                    op1=mybir.AluOpType.add,
                    accum_out=self.xsum4[:, t : t + 1],
                )
                self.xln.append(x_ln)
                junk = epi.tile([P, D], BF16, tag="junk")
                nc.scalar.activation(
                    out=junk,
                    in_=x_ln,
                    func=mybir.ActivationFunctionType.Square,
                    accum_out=self.sq4[:, t : t + 1],
                )

            def norm(self):
                # mean/var from raw moments (no bn_stats), then rsqrt as
                # exp(-0.5*ln(var+eps)): Ln and Exp live in the same ACT
                # table set, so the exp stream never swaps tables (Sqrt
                # lives in a different set and would thrash)
                self.m4 = epi.tile([P, 4], F32, tag="m4")
                nc.vector.tensor_scalar(
                    out=self.m4, in0=self.xsum4, scalar1=1.0 / D, scalar2=None,
                    op0=mybir.AluOpType.mult,
                )
                mm4 = epi.tile([P, 4], F32, tag="mm4")
                nc.vector.tensor_mul(mm4, self.m4, self.m4)
                v4 = epi.tile([P, 4], F32, tag="v4")
                nc.vector.scalar_tensor_tensor(
                    out=v4,
                    in0=self.sq4,
                    scalar=1.0 / D,
                    in1=mm4,
                    op0=mybir.AluOpType.mult,
                    op1=mybir.AluOpType.subtract,
                )
                lnv = epi.tile([P, 4], F32, tag="lnv")
                nc.scalar.activation(
                    out=lnv,
                    in_=v4,
                    func=mybir.ActivationFunctionType.Ln,
                    bias=eps_t[:, 0:1],
                    scale=1.0,
                )
                self.rs4 = epi.tile([P, 4], F32, tag="rs4")
                nc.scalar.activation(
                    out=self.rs4,
                    in_=lnv,
                    func=mybir.ActivationFunctionType.Exp,
                    bias=0.0,
                    scale=-0.5,
                )

            def pass2(self, t):
                t_sb = epi.tile([P, D], BF16, tag="t_sb")
                nc.vector.tensor_scalar(
                    t_sb,
                    self.xln[t],
                    self.m4[:, t : t + 1],
                    self.rs4[:, t : t + 1],
                    mybir.AluOpType.subtract,
                    mybir.AluOpType.mult,
                )
                g_sb = epi.tile([P, D], BF16, tag="g_sb")
                nc.gpsimd.tensor_mul(g_sb, t_sb, gamma_b16)
                o_sb = epi.tile([P, D], BF16, tag="o_sb")
                nc.vector.tensor_add(o_sb, g_sb, beta_b16)
                r0 = (self.oc * 4 + t) * P
                nc.sync.dma_start(out=out_ext[r0 : r0 + P, :], in_=o_sb)

        class OcState:
            def __init__(self, oc):
                self.oc = oc
                self.ctx_ps = ctxps_pool.tile([P, EC, 512], F32, tag="ctxps")
                self.s_ps = spsum.tile([1, 512], F32, tag="spsum")
                self.pt2 = None

        def emit_ib(st, ib):
            oc = st.oc
            if ib == 0 and oc + 1 < OC:
                for q in range(4):
                    load_e((oc + 1) * 4 + q)
            e8, il = e8_all[oc * 4 + ib // 4], ib % 4
            sp = sps.tile([P, 512], F32, tag="sps")
            for s in range(4):
                # start=True clears the whole PSUM bank -> only on s==0;
                # later mask MMs hit has_written=0 and write directly.
                nc.tensor.matmul(
                    out=sp[:, s * P : (s + 1) * P],
                    lhsT=e8[:, s, il * P : (il + 1) * P],
                    rhs=maskid8,
                    start=(s == 0),
                    stop=False,
                    skip_group_check=True,
                )
            for dc in (0, 2):
                nc.tensor.matmul(
                    out=sp,
                    lhsT=kt8[:, dc : dc + 2, ib * P : (ib + 1) * P],
                    rhs=qt8[:, dc : dc + 2, oc * 512 : (oc + 1) * 512],
                    start=False,
                    stop=(dc == 2),
                    perf_mode=mybir.MatmulPerfMode.DoubleRow,
                    skip_group_check=True,
                )
            if ib % 2 == 0:
                st.pt2 = ptp.tile([P, 2, 512], FP8, tag="pt")
            nc.scalar.activation(
                out=st.pt2[:, ib % 2, :],
                in_=sp,
                func=mybir.ActivationFunctionType.Exp,
                bias=cbias_t[:, 0:1],
                scale=ALPHA,
            )
            if ib % 2 == 1:
                j = (ib % 4) - 1
                for ec in range(EC):
                    nc.tensor.matmul(
                        out=st.ctx_ps[:, ec, :],
                        lhsT=x8g[ib // 4][:, j : j + 2, ec * P : (ec + 1) * P],
                        rhs=st.pt2,
                        start=(ib == 1),
                        stop=(ib == NB - 1),
                        perf_mode=mybir.MatmulPerfMode.DoubleRow,
                        skip_group_check=True,
                    )
                nc.tensor.matmul(
                    out=st.s_ps,
                    lhsT=ones8[:, :, 0:1],
                    rhs=st.pt2,
                    start=(ib == 1),
                    stop=(ib == NB - 1),
                    perf_mode=mybir.MatmulPerfMode.DoubleRow,
                    skip_group_check=True,
                )

        # oc0's i-blocks interleave with per-group prep: block 4g..4g+3
        # only needs group g's projections (subtile deps) + e-quarter g,
        # so the PE never sits idle waiting for the x DMA stream
        st = OcState(0)
        for g in range(4):
            prep_group(g)
            for ib in range(4 * g, 4 * g + 4):
                emit_ib(st, ib)

        pending = Epi(0, st.ctx_ps, st.s_ps)
        for oc in range(1, OC):
            st = OcState(oc)
            for ib in range(NB):
                emit_ib(st, ib)
                if ib == 0:
                    pending.head()
                elif ib == 1:
                    pending.scol()
                elif ib in (2, 4, 6, 8):
                    pending.wc(ib // 2 - 1)
                elif ib == 9:
                    pending.norm()
                elif ib in (10, 11, 12, 13):
                    # finish the DVE chain well before the oc ends so
                    # the next boundary's ctx8 casts start instantly
                    pending.pass2(ib - 10)
            pending = Epi(oc, st.ctx_ps, st.s_ps)

        # drain the final o-chunk's epilogue
        pending.head()
        pending.scol()
        for t in range(4):
            pending.wc(t)
        pending.norm()
        for t in range(4):
            pending.pass2(t)

    _split_excess_waits(nc)
    return nc


_NC_CACHE = None


def kernel(**inputs) -> np.ndarray:
    global _NC_CACHE
    _apply_patches()
    from concourse.bass_utils import run_bass_kernel_spmd

    node_fts = np.ascontiguousarray(np.asarray(inputs["node_fts"], dtype=np.float32))
    rel_edges = np.ascontiguousarray(np.asarray(inputs["rel_edges"], dtype=np.float32))
    shared = {
        k: np.ascontiguousarray(np.asarray(inputs[k], dtype=np.float32))
        for k in ("Wq", "bq", "Wk", "bk", "Wc", "gamma", "beta")
    }
    if _NC_CACHE is None:
        _NC_CACHE = build_nc()
    in_maps = [
        {"node_fts": node_fts[b], "rel_edges": rel_edges[b], **shared}
        for b in range(B)
    ]
    res = run_bass_kernel_spmd(_NC_CACHE, in_maps, core_ids=list(range(B)))
    return np.stack([res.results[b]["out"] for b in range(B)]).astype(np.float32)


# revision 36
# speedup vs baseline: 1.0170x; 1.0170x over previous
"""AttnGCN layer on 8 TRN2 NeuronCores — data-parallel over batch.

Per-core (one sample b):
  q = x @ Wq + bq ; k = x @ Wk + bk            (fp8 PE matmuls)
  sT[i,o] = k_i . q_o  + C'*eT[i,o]            (scores transposed; mask folded
                                                into PSUM via lhsT=e-block
                                                matmuls against a scaled
                                                identity — transposes e free)
  pT = exp(alpha*sT - C)                        (ACT, masked entries -> ~0)
  S[o] = sum_i pT[i,o]                          (ones-vector PE matmul)
  ctxT[e,o] = sum_i x8[i,e] * pT[i,o]           (PE, accumulated over i-blocks)
  P[o,c] = ctxT@Wc + S*x  ... LN(P) == LN(x + softmax@x@Wc) exactly
                                                (LayerNorm is scale-invariant
                                                 per row -> 1/S never needed)
  out = LN(P)*gamma + beta                      (DVE bn_stats/bn_aggr epilogue)

Self-contained: hardcodes shapes from the problem spec.
"""

import math
from contextlib import ExitStack

import numpy as np

import concourse.bass as bass
import concourse.tile as tile
from concourse import mybir
from concourse.vector_clock import ScopedClock

F32 = mybir.dt.float32
F32R = mybir.dt.float32r
BF16 = mybir.dt.bfloat16
FP8 = mybir.dt.float8e4

B = 8
N = 2048
D = 512
P = 128
NB = N // P       # 16 i-blocks
EC = D // P       # 4 chunks of the embed/dff dim
OC = N // 512     # 4 o-chunks of 512 attn rows
ALPHA = 1.0 / math.sqrt(D)
CPRIME = 128.0            # mask scale inside PSUM (fp8e4 max finite is 240)
SHIFT = 3.0               # softmax-invariant shift keeping exp() in fp8 range
CBIAS = CPRIME * ALPHA + SHIFT  # subtracted in the exp bias


# ---------------------------------------------------------------------------
# Workaround: walrus CoreV3 rejects >2 sem waits on the TileContext final
# drain ("Too many sync wait commands"). Hoist waits onto preceding nops.
def _patched_drain_and_barrier(self, tick_clock, wait_clock):
    nc = self.nc
    carrier = nc.sync.nop(nofuse=True)
    wait_clock.add_sem_waits(carrier.ins, ScopedClock({None: tick_clock.global_clock}))
    si = carrier.ins.sync_info
    waits = list(si.on_wait) if si and si.on_wait else []
    if len(waits) > 1:
        si.on_wait = waits[:1]
        for w in waits[1:]:
            n2 = nc.sync.nop(nofuse=True)
            n2.ins.sync_info = mybir.SyncInfo(on_wait=[w], on_update=[])
    nc.sync.drain()
    nc.all_engine_barrier()
    assert self.sems is not None
    popped = nc._tile_sem_poison_stack.pop()
    assert popped is self._sem_poison
    nc.clear_and_free_semaphores(list(self.sems.allocated().values()))
    nc.all_engine_barrier()


def _apply_patches():
    tile.TileContext._drain_and_barrier = _patched_drain_and_barrier


def _split_excess_waits(nc, limit=1):
    """walrus CoreV2/V3 codegen rejects instructions with >2 sem waits;
    hoist excess waits onto same-engine no-ops inserted just before."""
    n = 0
    for fn in nc.m.functions:
        for blk in fn.blocks:
            out = []
            changed = False
            for inst in blk.instructions:
                si = inst.sync_info
                waits = list(si.on_wait) if si and si.on_wait else []
                if len(waits) > limit:
                    keep = waits[-limit:]
                    for w in waits[:-limit]:
                        n += 1
                        nop = mybir.InstNoOp(name=f"I-wsplit-{n}", ins=[], outs=[])
                        nop.engine = inst.engine
                        nop.sync_info = mybir.SyncInfo(on_wait=[w], on_update=[])
                        out.append(nop)
                    si.on_wait = keep
                    changed = True
                out.append(inst)
            if changed:
                blk.instructions = out
    return n


def _identity(nc, ap, diag):
    nc.gpsimd.memset(ap, 0.0)
    nc.gpsimd.affine_select(
        out=ap,
        in_=ap,
        compare_op=mybir.AluOpType.not_equal,
        fill=diag,
        base=0,
        pattern=[[-1, ap.shape[0]]],
        channel_multiplier=1,
    )


def build_nc():
    nc = bass.Bass()
    x_ext = nc.declare_dram_parameter("node_fts", [N, D], F32, isOutput=False)
    e_ext = nc.declare_dram_parameter("rel_edges", [N, N], F32, isOutput=False)
    wq_ext = nc.declare_dram_parameter("Wq", [D, D], F32, isOutput=False)
    bq_ext = nc.declare_dram_parameter("bq", [D], F32, isOutput=False)
    wk_ext = nc.declare_dram_parameter("Wk", [D, D], F32, isOutput=False)
    bk_ext = nc.declare_dram_parameter("bk", [D], F32, isOutput=False)
    wc_ext = nc.declare_dram_parameter("Wc", [D, D], F32, isOutput=False)
    g_ext = nc.declare_dram_parameter("gamma", [D], F32, isOutput=False)
    be_ext = nc.declare_dram_parameter("beta", [D], F32, isOutput=False)
    out_ext = nc.declare_dram_parameter("out", [N, D], BF16, isOutput=True)

    with tile.TileContext(nc) as tc, ExitStack() as ctx:
        singles = ctx.enter_context(tc.tile_pool(name="singles", bufs=1))
        wstage = ctx.enter_context(tc.tile_pool(name="wstage", bufs=2))
        xtp = ctx.enter_context(tc.tile_pool(name="xtp", bufs=2))
        ptp = ctx.enter_context(tc.tile_pool(name="ptp", bufs=4))
        ctxp = ctx.enter_context(tc.tile_pool(name="ctxp", bufs=2))
        rowp = ctx.enter_context(tc.tile_pool(name="rowp", bufs=2))
        epi = ctx.enter_context(tc.tile_pool(name="epi", bufs=2))
        xlnp = ctx.enter_context(tc.tile_pool(name="xlnp", bufs=5))
        sps = ctx.enter_context(tc.tile_pool(name="sps", bufs=3, space="PSUM"))
        ctxps_pool = ctx.enter_context(tc.tile_pool(name="ctxps", bufs=1, space="PSUM"))
        spsum = ctx.enter_context(tc.tile_pool(name="spsum", bufs=1, space="PSUM"))

        # ---- persistent tiles -------------------------------------------
        xs_tiles = [
            singles.tile([P, 4, D], F32, tag=f"xs{g}", name=f"xs{g}")
            for g in range(4)
        ]
        x8g = [
            singles.tile([P, 4, D], FP8, tag=f"x8g{g}", name=f"x8g{g}")
            for g in range(4)
        ]
        # 16 e-quarters, fp8, each [128 (o in block), 4 (o blocks), 512 (i)]
        e8_all = [
            singles.tile([P, 4, 512], FP8, tag=f"e8_{j}", name=f"e8_{j}")
            for j in range(16)
        ]
        qt8 = singles.tile([P, EC, N], FP8, tag="qt8")
        kt8 = singles.tile([P, EC, N], FP8, tag="kt8")
        wq8 = singles.tile([P, EC, D], FP8, tag="wq8")
        wk8 = singles.tile([P, EC, D], FP8, tag="wk8")
        wc8 = singles.tile([P, EC, D], FP8, tag="wc8")
        bq_row = singles.tile([1, D], F32, tag="bq_row")
        bk_row = singles.tile([1, D], F32, tag="bk_row")
        g_row = singles.tile([1, D], F32, tag="g_row")
        be_row = singles.tile([1, D], F32, tag="be_row")
        bqkt = singles.tile([P, 2 * EC], F32, tag="bqkt")
        gamma_b16 = singles.tile([P, D], BF16, tag="gamma_b16")
        beta_b16 = singles.tile([P, D], BF16, tag="beta_b16")
        ident32 = singles.tile([P, P], F32, tag="ident32")
        maskid8 = singles.tile([P, P], FP8, tag="maskid8")
        ones8 = singles.tile([P, 2, 16], FP8, tag="ones8")
        one32 = singles.tile([1, 1], F32, tag="one32")
        eps_t = singles.tile([P, 1], F32, tag="eps_t")
        cbias_t = singles.tile([P, 1], F32, tag="cbias_t")

        # gpsimd queue: identity build only, then DMA triggers ASAP;
        # the rest of the constants go on DVE so the SWDGE loads start early
        _identity(nc, ident32, 1.0)
        nc.vector.tensor_scalar(
            out=maskid8, in0=ident32, scalar1=CPRIME, scalar2=None,
            op0=mybir.AluOpType.mult,
        )
        nc.vector.memset(ones8, 1.0)
        nc.vector.memset(one32, 1.0)
        nc.vector.memset(eps_t, 1e-5)
        nc.vector.memset(cbias_t, -CBIAS)

        # ---- input loads, split across the three DMA paths --------------
        # sync+scalar HWDGE: x (f32, prep-critical; the ~1us trigger cost
        # per DMA serializes on one engine); SWDGE (casting): weights->fp8
        # then edges->fp8. Non-critical loads get tile_wait_until so the
        # priority scheduler doesn't float them into the x stream's window
        # (dep-free DMA triggers otherwise all issue at t~0 and the SDMA
        # round-robin starves x: xs3 was landing at 29-33us).
        def load_xs(g):
            eng = nc.sync if g % 2 == 0 else nc.scalar
            eng.dma_start(
                out=xs_tiles[g],
                in_=x_ext[g * 4 * P : (g + 1) * 4 * P, :].rearrange(
                    "(ib p) e -> p ib e", p=P
                ),
            )

        def load_e(j):  # j = oc*4 + q
            oc, q = j // 4, j % 4
            nc.gpsimd.dma_start(
                out=e8_all[j],
                in_=e_ext[
                    oc * 512 : (oc + 1) * 512, q * 512 : (q + 1) * 512
                ].rearrange("(s p) f -> p s f", p=P),
            )

        def load_w(w_ext, w8):
            nc.gpsimd.dma_start(
                out=w8, in_=w_ext[:, :].rearrange("(ec p) f -> p ec f", p=P)
            )

        # prep-critical loads only; e-quarters for oc>0 are issued inside the
        # main loop (one oc ahead) so they don't steal DMA bandwidth from x/w
        for g in range(4):
            load_xs(g)
        load_w(wq_ext, wq8)
        load_w(wk_ext, wk8)
        for j, ms in enumerate((0.012, 0.016, 0.020, 0.024)):
            with tc.tile_wait_until(ms):
                load_e(j)
        with tc.tile_wait_until(0.028):
            load_w(wc_ext, wc8)

        # ---- HAM warmup: dummy matmul burst while the first DMAs land ----
        # PE clock-gate needs ~3.4us of sustained activity to go 1.2->2.4GHz;
        # burn the DMA-wait with throwaway matmuls so prep runs warm. The
        # operand is an uninitialized tile (no writer -> no deps) so the
        # burst starts right after the NEFF preamble, before the constants.
        warm_src = singles.tile([P, P], FP8, tag="warm_src")
        nc.vector.memset(warm_src, 1.0)
        warm_ps = sps.tile([P, 512], F32, tag="sps")
        for j in range(72):
            nc.tensor.matmul(
                out=warm_ps[:, (j % 4) * P : (j % 4 + 1) * P],
                lhsT=warm_src,
                rhs=warm_src,
                start=True,
                stop=True,
                skip_group_check=True,
            )

        # vector params as single-row loads (1 descriptor each; the old
        # [128,D] broadcast DMAs generated 4-byte descriptors and cost
        # 8.3us of engine time per trigger). Broadcast/transpose on-chip.
        def _row(ap_1d):
            return bass.AP(
                tensor=ap_1d.tensor, offset=ap_1d.offset, ap=[[0, 1], *ap_1d.ap]
            )

        with tc.tile_wait_until(0.006):
            nc.scalar.dma_start(out=bq_row, in_=_row(bq_ext[:]))
            nc.scalar.dma_start(out=bk_row, in_=_row(bk_ext[:]))
            nc.scalar.dma_start(out=g_row, in_=_row(g_ext[:]))
            nc.scalar.dma_start(out=be_row, in_=_row(be_ext[:]))

        # broadcast rows across partitions via a K=1 PE matmul
        # (ones_col.T @ row), then one cast each to bf16
        ones_col = singles.tile([1, P], F32, tag="ones_col")
        nc.vector.memset(ones_col, 1.0)
        for row, dst in ((g_row, gamma_b16), (be_row, beta_b16)):
            bc_ps = sps.tile([P, 512], F32, tag="sps")
            nc.tensor.matmul(
                out=bc_ps,
                lhsT=ones_col,
                rhs=row,
                start=True,
                stop=True,
                skip_group_check=True,
            )
            nc.vector.tensor_copy(out=dst, in_=bc_ps)

        # biases per-partition [p, fc] (f = fc*128 + p) via tiny PE
        # transposes, like the S column later
        bqk_ps = sps.tile([P, 2 * EC], F32, tag="sps")
        for j in range(8):
            src = bq_row if j < 4 else bk_row
            nc.tensor.matmul(
                out=bqk_ps[:, j : j + 1],
                lhsT=src[0:1, (j % 4) * P : (j % 4 + 1) * P],
                rhs=one32,
                is_transpose=True,
                start=(j == 0),
                stop=(j == 7),
                skip_group_check=True,
            )
        nc.vector.tensor_copy(out=bqkt, in_=bqk_ps)
        bqt = bqkt[:, 0:EC]
        bkt = bqkt[:, EC : 2 * EC]

        # ---- per-group prep: x8 cast, transposes, q/k projections -------
        def prep_group(g):
            xs = xs_tiles[g]
            nc.vector.tensor_copy(out=x8g[g], in_=xs)
            xt = xtp.tile([P, EC, 512], FP8, tag="xt")
            for ec in range(EC):
                tp = sps.tile([P, 512], F32, tag="sps")
                for k4 in range(4):
                    nc.tensor.transpose(
                        out=tp[:, k4 * P : (k4 + 1) * P],
                        in_=xs[:, k4, ec * P : (ec + 1) * P],
                        identity=ident32,
                    )
                nc.vector.tensor_copy(out=xt[:, ec, :], in_=tp)
            for w8, bt, dst in ((wq8, bqt, qt8), (wk8, bkt, kt8)):
                for fc in range(EC):
                    ps = sps.tile([P, 512], F32, tag="sps")
                    for dc in (0, 2):
                        nc.tensor.matmul(
                            out=ps,
                            lhsT=w8[:, dc : dc + 2, fc * P : (fc + 1) * P],
                            rhs=xt[:, dc : dc + 2, :],
                            start=(dc == 0),
                            stop=(dc == 2),
                            perf_mode=mybir.MatmulPerfMode.DoubleRow,
                            skip_group_check=True,
                        )
                    # bias + fp8 cast: q on ACT, k on DVE — splitting the
                    # PSUM->SBUF drain keeps either engine off the prep
                    # critical path (ACT alone was the prep bottleneck)
                    if dst is qt8:
                        nc.scalar.activation(
                            out=dst[:, fc, g * 512 : (g + 1) * 512],
                            in_=ps,
                            func=mybir.ActivationFunctionType.Identity,
                            bias=bt[:, fc : fc + 1],
                            scale=1.0,
                        )
                    else:
                        nc.vector.tensor_scalar(
                            out=dst[:, fc, g * 512 : (g + 1) * 512],
                            in0=ps,
                            scalar1=bt[:, fc : fc + 1],
                            scalar2=None,
                            op0=mybir.AluOpType.add,
                        )

        # ---- main loop over o-chunks ------------------------------------
        # The per-oc epilogue is software-pipelined into the NEXT oc's
        # i-block loop: the PE engine queue is in-order, so emitting
        # epilogue PE work (s_col transposes, Wc matmuls) contiguously at
        # the oc boundary head-of-line blocks ready score matmuls behind a
        # DVE dependency chain. Interleaving stages at fixed i-blocks keeps
        # PE dense and spreads the DVE/gpsimd chain across the whole oc.
        class Epi:
            def __init__(self, oc, ctx_ps, s_ps):
                self.oc = oc
                self.ctx_ps = ctx_ps
                self.s_ps = s_ps
                self.xln = []

            def head(self):
                # free the S row bank + ctx_ps banks ASAP; split the copies
                # across ACT and DVE so neither queue's backlog delays the
                # next oc's first ctx matmul
                self.s_sb = rowp.tile([1, 512], F32, tag="s_sb")
                nc.scalar.copy(out=self.s_sb, in_=self.s_ps)
                self.ctx8 = ctxp.tile([P, EC, 512], FP8, tag="ctx8")
                for ec in range(EC):
                    eng_copy = (
                        nc.vector.tensor_copy if ec < 2 else nc.scalar.copy
                    )
                    eng_copy(out=self.ctx8[:, ec, :], in_=self.ctx_ps[:, ec, :])

            def scol(self):
                # S per-partition via tiny PE transposes (no reciprocal:
                # LayerNorm is scale-invariant per row, so we feed it
                # P = ctx_unnorm@Wc + S*x instead of x + (ctx_unnorm/S)@Wc)
                s_col = sps.tile([P, 4], F32, tag="sps")
                for j in range(4):
                    nc.tensor.matmul(
                        out=s_col[:, j : j + 1],
                        lhsT=self.s_sb[0:1, j * P : (j + 1) * P],
                        rhs=one32,
                        is_transpose=True,
                        start=(j == 0),
                        stop=(j == 3),
                        skip_group_check=True,
                    )
                self.s_colv = rowp.tile([P, 4], F32, tag="s_colv")
                nc.vector.tensor_copy(out=self.s_colv, in_=s_col)
                self.xsum4 = epi.tile([P, 4], F32, tag="xsum4")
                self.sq4 = epi.tile([P, 4], F32, tag="sq4")

            def wc(self, t):
                opre = sps.tile([P, 512], F32, tag="sps")
                for dc in (0, 2):
                    nc.tensor.matmul(
                        out=opre,
                        lhsT=self.ctx8[:, dc : dc + 2, t * P : (t + 1) * P],
                        rhs=wc8[:, dc : dc + 2, :],
                        start=(dc == 0),
                        stop=(dc == 2),
                        perf_mode=mybir.MatmulPerfMode.DoubleRow,
                        skip_group_check=True,
                    )
                # x_ln = S*xs + ctx@Wc, with sum(x_ln) accumulated for free;
                # the second moment runs on gpsimd so the DVE never does
                # bn_stats (keeps the DVE queue short at oc boundaries)
                x_ln = xlnp.tile([P, D], F32, tag="x_ln")
                nc.vector.scalar_tensor_tensor(
                    out=x_ln,
                    in0=xs_tiles[self.oc][:, t, :],
                    scalar=self.s_colv[:, t : t + 1],
                    in1=opre,
                    op0=mybir.AluOpType.mult,
                    op1=mybir.AluOpType.add,
                    accum_out=self.xsum4[:, t : t + 1],
                )
                self.xln.append(x_ln)
                junk = epi.tile([P, D], BF16, tag="junk")
                nc.scalar.activation(
                    out=junk,
                    in_=x_ln,
                    func=mybir.ActivationFunctionType.Square,
                    accum_out=self.sq4[:, t : t + 1],
                )

            def norm(self):
                # mean/var from raw moments (no bn_stats), then rsqrt as
                # exp(-0.5*ln(var+eps)): Ln and Exp live in the same ACT
                # table set, so the exp stream never swaps tables (Sqrt
                # lives in a different set and would thrash)
                self.m4 = epi.tile([P, 4], F32, tag="m4")
                nc.vector.tensor_scalar(
                    out=self.m4, in0=self.xsum4, scalar1=1.0 / D, scalar2=None,
                    op0=mybir.AluOpType.mult,
                )
                mm4 = epi.tile([P, 4], F32, tag="mm4")
                nc.vector.tensor_mul(mm4, self.m4, self.m4)
                v4 = epi.tile([P, 4], F32, tag="v4")
                nc.vector.scalar_tensor_tensor(
                    out=v4,
                    in0=self.sq4,
                    scalar=1.0 / D,
                    in1=mm4,
                    op0=mybir.AluOpType.mult,
                    op1=mybir.AluOpType.subtract,
                )
                lnv = epi.tile([P, 4], F32, tag="lnv")
                nc.scalar.activation(
                    out=lnv,
                    in_=v4,
                    func=mybir.ActivationFunctionType.Ln,
                    bias=eps_t[:, 0:1],
                    scale=1.0,
                )
                self.rs4 = epi.tile([P, 4], F32, tag="rs4")
                nc.scalar.activation(
                    out=self.rs4,
                    in_=lnv,
                    func=mybir.ActivationFunctionType.Exp,
                    bias=0.0,
                    scale=-0.5,
                )

            def pass2(self, t):
                t_sb = epi.tile([P, D], BF16, tag="t_sb")
                nc.vector.tensor_scalar(
                    t_sb,
                    self.xln[t],
                    self.m4[:, t : t + 1],
                    self.rs4[:, t : t + 1],
                    mybir.AluOpType.subtract,
                    mybir.AluOpType.mult,
                )
                g_sb = epi.tile([P, D], BF16, tag="g_sb")
                nc.gpsimd.tensor_mul(g_sb, t_sb, gamma_b16)
                o_sb = epi.tile([P, D], BF16, tag="o_sb")
                nc.vector.tensor_add(o_sb, g_sb, beta_b16)
                r0 = (self.oc * 4 + t) * P
                nc.sync.dma_start(out=out_ext[r0 : r0 + P, :], in_=o_sb)

        class OcState:
            def __init__(self, oc):
                self.oc = oc
                self.ctx_ps = ctxps_pool.tile([P, EC, 512], F32, tag="ctxps")
                self.s_ps = spsum.tile([1, 512], F32, tag="spsum")
                self.pt2 = None

        def emit_ib(st, ib):
            oc = st.oc
            if oc == 0:
                # oc0 runs interleaved with prep; spread oc1's edge loads
                # so they don't contend with the x/w/e0-3 prep stream
                if ib == 8:
                    load_e(4)
                    load_e(5)
                elif ib == 12:
                    load_e(6)
                    load_e(7)
            elif ib == 0 and oc + 1 < OC:
                for q in range(4):
                    load_e((oc + 1) * 4 + q)
            e8, il = e8_all[oc * 4 + ib // 4], ib % 4
            sp = sps.tile([P, 512], F32, tag="sps")
            for s in range(4):
                # start=True clears the whole PSUM bank -> only on s==0;
                # later mask MMs hit has_written=0 and write directly.
                nc.tensor.matmul(
                    out=sp[:, s * P : (s + 1) * P],
                    lhsT=e8[:, s, il * P : (il + 1) * P],
                    rhs=maskid8,
                    start=(s == 0),
                    stop=False,
                    skip_group_check=True,
                )
            for dc in (0, 2):
                nc.tensor.matmul(
                    out=sp,
                    lhsT=kt8[:, dc : dc + 2, ib * P : (ib + 1) * P],
                    rhs=qt8[:, dc : dc + 2, oc * 512 : (oc + 1) * 512],
                    start=False,
                    stop=(dc == 2),
                    perf_mode=mybir.MatmulPerfMode.DoubleRow,
                    skip_group_check=True,
                )
            if ib % 2 == 0:
                st.pt2 = ptp.tile([P, 2, 512], FP8, tag="pt")
            nc.scalar.activation(
                out=st.pt2[:, ib % 2, :],
                in_=sp,
                func=mybir.ActivationFunctionType.Exp,
                bias=cbias_t[:, 0:1],
                scale=ALPHA,
            )
            if ib % 2 == 1:
                j = (ib % 4) - 1
                for ec in range(EC):
                    nc.tensor.matmul(
                        out=st.ctx_ps[:, ec, :],
                        lhsT=x8g[ib // 4][:, j : j + 2, ec * P : (ec + 1) * P],
                        rhs=st.pt2,
                        start=(ib == 1),
                        stop=(ib == NB - 1),
                        perf_mode=mybir.MatmulPerfMode.DoubleRow,
                        skip_group_check=True,
                    )
                nc.tensor.matmul(
                    out=st.s_ps,
                    lhsT=ones8[:, :, 0:1],
                    rhs=st.pt2,
                    start=(ib == 1),
                    stop=(ib == NB - 1),
                    perf_mode=mybir.MatmulPerfMode.DoubleRow,
                    skip_group_check=True,
                )

        # oc0's i-blocks interleave with per-group prep, trailing by one
        # group: block 4g..4g+3 only needs group g's projections (subtile
        # deps) + e-quarter g, and trailing gives each e-quarter DMA ~11us
        # of slack so a late edge load never head-of-line blocks prep
        st = OcState(0)
        prep_group(0)
        for g in range(1, 4):
            prep_group(g)
            for ib in range(4 * (g - 1), 4 * g):
                emit_ib(st, ib)
        for ib in range(12, 16):
            emit_ib(st, ib)

        pending = Epi(0, st.ctx_ps, st.s_ps)
        for oc in range(1, OC):
            st = OcState(oc)
            for ib in range(NB):
                emit_ib(st, ib)
                if ib == 0:
                    pending.head()
                elif ib == 1:
                    pending.scol()
                elif ib in (2, 4, 6, 8):
                    pending.wc(ib // 2 - 1)
                elif ib == 9:
                    pending.norm()
                elif ib in (10, 11, 12, 13):
                    # finish the DVE chain well before the oc ends so
                    # the next boundary's ctx8 casts start instantly
                    pending.pass2(ib - 10)
            pending = Epi(oc, st.ctx_ps, st.s_ps)

        # drain the final o-chunk's epilogue
        pending.head()
        pending.scol()
        for t in range(4):
            pending.wc(t)
        pending.norm()
        for t in range(4):
            pending.pass2(t)

    _split_excess_waits(nc)
    return nc


_NC_CACHE = None


def kernel(**inputs) -> np.ndarray:
    global _NC_CACHE
    _apply_patches()
    from concourse.bass_utils import run_bass_kernel_spmd

    node_fts = np.ascontiguousarray(np.asarray(inputs["node_fts"], dtype=np.float32))
    rel_edges = np.ascontiguousarray(np.asarray(inputs["rel_edges"], dtype=np.float32))
    shared = {
        k: np.ascontiguousarray(np.asarray(inputs[k], dtype=np.float32))
        for k in ("Wq", "bq", "Wk", "bk", "Wc", "gamma", "beta")
    }
    if _NC_CACHE is None:
        _NC_CACHE = build_nc()
    in_maps = [
        {"node_fts": node_fts[b], "rel_edges": rel_edges[b], **shared}
        for b in range(B)
    ]
    res = run_bass_kernel_spmd(_NC_CACHE, in_maps, core_ids=list(range(B)))
    return np.stack([res.results[b]["out"] for b in range(B)]).astype(np.float32)


# revision 39
# speedup vs baseline: 1.0869x; 1.0687x over previous
"""AttnGCN layer on 8 TRN2 NeuronCores — data-parallel over batch.

Per-core (one sample b):
  q = x @ Wq + bq ; k = x @ Wk + bk            (fp8 PE matmuls)
  sT[i,o] = k_i . q_o  + C'*eT[i,o]            (scores transposed; mask folded
                                                into PSUM via lhsT=e-block
                                                matmuls against a scaled
                                                identity — transposes e free)
  pT = exp(alpha*sT - C)                        (ACT, masked entries -> ~0)
  S[o] = sum_i pT[i,o]                          (ones-vector PE matmul)
  ctxT[e,o] = sum_i x8[i,e] * pT[i,o]           (PE, accumulated over i-blocks)
  P[o,c] = ctxT@Wc + S*x  ... LN(P) == LN(x + softmax@x@Wc) exactly
                                                (LayerNorm is scale-invariant
                                                 per row -> 1/S never needed)
  out = LN(P)*gamma + beta                      (DVE bn_stats/bn_aggr epilogue)

Self-contained: hardcodes shapes from the problem spec.
"""

import math
from contextlib import ExitStack

import numpy as np

import concourse.bass as bass
import concourse.tile as tile
from concourse import mybir
from concourse.vector_clock import ScopedClock

F32 = mybir.dt.float32
F32R = mybir.dt.float32r
BF16 = mybir.dt.bfloat16
FP8 = mybir.dt.float8e4

B = 8
N = 2048
D = 512
P = 128
NB = N // P       # 16 i-blocks
EC = D // P       # 4 chunks of the embed/dff dim
OC = N // 512     # 4 o-chunks of 512 attn rows
ALPHA = 1.0 / math.sqrt(D)
CPRIME = 128.0            # mask scale inside PSUM (fp8e4 max finite is 240)
SHIFT = 3.0               # softmax-invariant shift keeping exp() in fp8 range
CBIAS = CPRIME * ALPHA + SHIFT  # subtracted in the exp bias


# ---------------------------------------------------------------------------
# Workaround: walrus CoreV3 rejects >2 sem waits on the TileContext final
# drain ("Too many sync wait commands"). Hoist waits onto preceding nops.
def _patched_drain_and_barrier(self, tick_clock, wait_clock):
    nc = self.nc
    carrier = nc.sync.nop(nofuse=True)
    wait_clock.add_sem_waits(carrier.ins, ScopedClock({None: tick_clock.global_clock}))
    si = carrier.ins.sync_info
    waits = list(si.on_wait) if si and si.on_wait else []
    if len(waits) > 1:
        si.on_wait = waits[:1]
        for w in waits[1:]:
            n2 = nc.sync.nop(nofuse=True)
            n2.ins.sync_info = mybir.SyncInfo(on_wait=[w], on_update=[])
    nc.sync.drain()
    nc.all_engine_barrier()
    assert self.sems is not None
    popped = nc._tile_sem_poison_stack.pop()
    assert popped is self._sem_poison
    nc.clear_and_free_semaphores(list(self.sems.allocated().values()))
    nc.all_engine_barrier()


def _apply_patches():
    tile.TileContext._drain_and_barrier = _patched_drain_and_barrier


def _split_excess_waits(nc, limit=1):
    """walrus CoreV2/V3 codegen rejects instructions with >2 sem waits;
    hoist excess waits onto same-engine no-ops inserted just before."""
    n = 0
    for fn in nc.m.functions:
        for blk in fn.blocks:
            out = []
            changed = False
            for inst in blk.instructions:
                si = inst.sync_info
                waits = list(si.on_wait) if si and si.on_wait else []
                if len(waits) > limit:
                    keep = waits[-limit:]
                    for w in waits[:-limit]:
                        n += 1
                        nop = mybir.InstNoOp(name=f"I-wsplit-{n}", ins=[], outs=[])
                        nop.engine = inst.engine
                        nop.sync_info = mybir.SyncInfo(on_wait=[w], on_update=[])
                        out.append(nop)
                    si.on_wait = keep
                    changed = True
                out.append(inst)
            if changed:
                blk.instructions = out
    return n


def _identity(nc, ap, diag):
    nc.gpsimd.memset(ap, 0.0)
    nc.gpsimd.affine_select(
        out=ap,
        in_=ap,
        compare_op=mybir.AluOpType.not_equal,
        fill=diag,
        base=0,
        pattern=[[-1, ap.shape[0]]],
        channel_multiplier=1,
    )


def build_nc():
    nc = bass.Bass()
    x_ext = nc.declare_dram_parameter("node_fts", [N, D], F32, isOutput=False)
    e_ext = nc.declare_dram_parameter("rel_edges", [N, N], F32, isOutput=False)
    wq_ext = nc.declare_dram_parameter("Wq", [D, D], F32, isOutput=False)
    bq_ext = nc.declare_dram_parameter("bq", [D], F32, isOutput=False)
    wk_ext = nc.declare_dram_parameter("Wk", [D, D], F32, isOutput=False)
    bk_ext = nc.declare_dram_parameter("bk", [D], F32, isOutput=False)
    wc_ext = nc.declare_dram_parameter("Wc", [D, D], F32, isOutput=False)
    g_ext = nc.declare_dram_parameter("gamma", [D], F32, isOutput=False)
    be_ext = nc.declare_dram_parameter("beta", [D], F32, isOutput=False)
    out_ext = nc.declare_dram_parameter("out", [N, D], BF16, isOutput=True)

    with tile.TileContext(nc) as tc, ExitStack() as ctx:
        singles = ctx.enter_context(tc.tile_pool(name="singles", bufs=1))
        wstage = ctx.enter_context(tc.tile_pool(name="wstage", bufs=2))
        xtp = ctx.enter_context(tc.tile_pool(name="xtp", bufs=2))
        ptp = ctx.enter_context(tc.tile_pool(name="ptp", bufs=4))
        ctxp = ctx.enter_context(tc.tile_pool(name="ctxp", bufs=2))
        rowp = ctx.enter_context(tc.tile_pool(name="rowp", bufs=2))
        epi = ctx.enter_context(tc.tile_pool(name="epi", bufs=2))
        xlnp = ctx.enter_context(tc.tile_pool(name="xlnp", bufs=5))
        sps = ctx.enter_context(tc.tile_pool(name="sps", bufs=3, space="PSUM"))
        ctxps_pool = ctx.enter_context(tc.tile_pool(name="ctxps", bufs=1, space="PSUM"))
        spsum = ctx.enter_context(tc.tile_pool(name="spsum", bufs=1, space="PSUM"))

        # ---- persistent tiles -------------------------------------------
        xs_tiles = [
            singles.tile([P, 4, D], F32, tag=f"xs{g}", name=f"xs{g}")
            for g in range(4)
        ]
        x8g = [
            singles.tile([P, 4, D], FP8, tag=f"x8g{g}", name=f"x8g{g}")
            for g in range(4)
        ]
        # 16 e-quarters, fp8, each [128 (o in block), 4 (o blocks), 512 (i)]
        e8_all = [
            singles.tile([P, 4, 512], FP8, tag=f"e8_{j}", name=f"e8_{j}")
            for j in range(16)
        ]
        qt8 = singles.tile([P, EC, N], FP8, tag="qt8")
        kt8 = singles.tile([P, EC, N], FP8, tag="kt8")
        wq8 = singles.tile([P, EC, D], FP8, tag="wq8")
        wk8 = singles.tile([P, EC, D], FP8, tag="wk8")
        wc8 = singles.tile([P, EC, D], FP8, tag="wc8")
        bq_row = singles.tile([1, D], F32, tag="bq_row")
        bk_row = singles.tile([1, D], F32, tag="bk_row")
        g_row = singles.tile([1, D], F32, tag="g_row")
        be_row = singles.tile([1, D], F32, tag="be_row")
        bqkt = singles.tile([P, 2 * EC], F32, tag="bqkt")
        gamma_b16 = singles.tile([P, D], BF16, tag="gamma_b16")
        beta_b16 = singles.tile([P, D], BF16, tag="beta_b16")
        ident32 = singles.tile([P, P], F32, tag="ident32")
        maskid8 = singles.tile([P, P], FP8, tag="maskid8")
        ones8 = singles.tile([P, 2, 16], FP8, tag="ones8")
        one32 = singles.tile([1, 1], F32, tag="one32")
        eps_t = singles.tile([P, 1], F32, tag="eps_t")
        cbias_t = singles.tile([P, 1], F32, tag="cbias_t")

        # gpsimd queue: identity build only, then DMA triggers ASAP;
        # the rest of the constants go on DVE so the SWDGE loads start early
        _identity(nc, ident32, 1.0)
        nc.vector.tensor_scalar(
            out=maskid8, in0=ident32, scalar1=CPRIME, scalar2=None,
            op0=mybir.AluOpType.mult,
        )
        nc.vector.memset(ones8, 1.0)
        nc.vector.memset(one32, 1.0)
        nc.vector.memset(eps_t, 1e-5)
        nc.vector.memset(cbias_t, -CBIAS)

        # ---- input loads, split across the three DMA paths --------------
        # sync+scalar HWDGE: x (f32, prep-critical; the ~1us trigger cost
        # per DMA serializes on one engine); SWDGE (casting): weights->fp8
        # then edges->fp8. Non-critical loads get tile_wait_until so the
        # priority scheduler doesn't float them into the x stream's window
        # (dep-free DMA triggers otherwise all issue at t~0 and the SDMA
        # round-robin starves x: xs3 was landing at 29-33us).
        def load_xs(g):
            eng = nc.sync if g % 2 == 0 else nc.scalar
            eng.dma_start(
                out=xs_tiles[g],
                in_=x_ext[g * 4 * P : (g + 1) * 4 * P, :].rearrange(
                    "(ib p) e -> p ib e", p=P
                ),
            )

        def load_e(j):  # j = oc*4 + q
            oc, q = j // 4, j % 4
            nc.gpsimd.dma_start(
                out=e8_all[j],
                in_=e_ext[
                    oc * 512 : (oc + 1) * 512, q * 512 : (q + 1) * 512
                ].rearrange("(s p) f -> p s f", p=P),
            )

        def load_w(w_ext, w8):
            nc.gpsimd.dma_start(
                out=w8, in_=w_ext[:, :].rearrange("(ec p) f -> p ec f", p=P)
            )

        # prep-critical loads only; e-quarters for oc>0 are issued inside the
        # main loop (one oc ahead) so they don't steal DMA bandwidth from x/w
        for g in range(4):
            load_xs(g)
        load_w(wq_ext, wq8)
        load_w(wk_ext, wk8)
        for j, ms in enumerate((0.010, 0.012, 0.014, 0.016)):
            with tc.tile_wait_until(ms):
                load_e(j)
        with tc.tile_wait_until(0.018):
            load_w(wc_ext, wc8)

        # ---- HAM warmup: dummy matmul burst while the first DMAs land ----
        # PE clock-gate needs ~3.4us of sustained activity to go 1.2->2.4GHz;
        # burn the DMA-wait with throwaway matmuls so prep runs warm. The
        # operand is an uninitialized tile (no writer -> no deps) so the
        # burst starts right after the NEFF preamble, before the constants.
        warm_src = singles.tile([P, P], FP8, tag="warm_src")
        nc.vector.memset(warm_src, 1.0)
        warm_ps = sps.tile([P, 512], F32, tag="sps")
        for j in range(72):
            nc.tensor.matmul(
                out=warm_ps[:, (j % 4) * P : (j % 4 + 1) * P],
                lhsT=warm_src,
                rhs=warm_src,
                start=True,
                stop=True,
                skip_group_check=True,
            )

        # vector params as single-row loads (1 descriptor each; the old
        # [128,D] broadcast DMAs generated 4-byte descriptors and cost
        # 8.3us of engine time per trigger). Broadcast/transpose on-chip.
        def _row(ap_1d):
            return bass.AP(
                tensor=ap_1d.tensor, offset=ap_1d.offset, ap=[[0, 1], *ap_1d.ap]
            )

        with tc.tile_wait_until(0.006):
            nc.scalar.dma_start(out=bq_row, in_=_row(bq_ext[:]))
            nc.scalar.dma_start(out=bk_row, in_=_row(bk_ext[:]))
            nc.scalar.dma_start(out=g_row, in_=_row(g_ext[:]))
            nc.scalar.dma_start(out=be_row, in_=_row(be_ext[:]))

        # broadcast rows across partitions via a K=1 PE matmul
        # (ones_col.T @ row), then one cast each to bf16
        ones_col = singles.tile([1, P], F32, tag="ones_col")
        nc.vector.memset(ones_col, 1.0)
        for row, dst in ((g_row, gamma_b16), (be_row, beta_b16)):
            bc_ps = sps.tile([P, 512], F32, tag="sps")
            nc.tensor.matmul(
                out=bc_ps,
                lhsT=ones_col,
                rhs=row,
                start=True,
                stop=True,
                skip_group_check=True,
            )
            nc.vector.tensor_copy(out=dst, in_=bc_ps)

        # biases per-partition [p, fc] (f = fc*128 + p) via tiny PE
        # transposes, like the S column later
        bqk_ps = sps.tile([P, 2 * EC], F32, tag="sps")
        for j in range(8):
            src = bq_row if j < 4 else bk_row
            nc.tensor.matmul(
                out=bqk_ps[:, j : j + 1],
                lhsT=src[0:1, (j % 4) * P : (j % 4 + 1) * P],
                rhs=one32,
                is_transpose=True,
                start=(j == 0),
                stop=(j == 7),
                skip_group_check=True,
            )
        nc.vector.tensor_copy(out=bqkt, in_=bqk_ps)
        bqt = bqkt[:, 0:EC]
        bkt = bqkt[:, EC : 2 * EC]

        # ---- per-group prep: x8 cast, transposes, q/k projections -------
        def prep_group(g):
            xs = xs_tiles[g]
            nc.vector.tensor_copy(out=x8g[g], in_=xs)
            xt = xtp.tile([P, EC, 512], FP8, tag="xt")
            for ec in range(EC):
                tp = sps.tile([P, 512], F32, tag="sps")
                for k4 in range(4):
                    nc.tensor.transpose(
                        out=tp[:, k4 * P : (k4 + 1) * P],
                        in_=xs[:, k4, ec * P : (ec + 1) * P],
                        identity=ident32,
                    )
                nc.vector.tensor_copy(out=xt[:, ec, :], in_=tp)
            for w8, bt, dst in ((wq8, bqt, qt8), (wk8, bkt, kt8)):
                for fc in range(EC):
                    ps = sps.tile([P, 512], F32, tag="sps")
                    for dc in (0, 2):
                        nc.tensor.matmul(
                            out=ps,
                            lhsT=w8[:, dc : dc + 2, fc * P : (fc + 1) * P],
                            rhs=xt[:, dc : dc + 2, :],
                            start=(dc == 0),
                            stop=(dc == 2),
                            perf_mode=mybir.MatmulPerfMode.DoubleRow,
                            skip_group_check=True,
                        )
                    # bias + fp8 cast: q on ACT, k on DVE — splitting the
                    # PSUM->SBUF drain keeps either engine off the prep
                    # critical path (ACT alone was the prep bottleneck)
                    if dst is qt8:
                        nc.scalar.activation(
                            out=dst[:, fc, g * 512 : (g + 1) * 512],
                            in_=ps,
                            func=mybir.ActivationFunctionType.Identity,
                            bias=bt[:, fc : fc + 1],
                            scale=1.0,
                        )
                    else:
                        nc.vector.tensor_scalar(
                            out=dst[:, fc, g * 512 : (g + 1) * 512],
                            in0=ps,
                            scalar1=bt[:, fc : fc + 1],
                            scalar2=None,
                            op0=mybir.AluOpType.add,
                        )

        # ---- main loop over o-chunks ------------------------------------
        # The per-oc epilogue is software-pipelined into the NEXT oc's
        # i-block loop: the PE engine queue is in-order, so emitting
        # epilogue PE work (s_col transposes, Wc matmuls) contiguously at
        # the oc boundary head-of-line blocks ready score matmuls behind a
        # DVE dependency chain. Interleaving stages at fixed i-blocks keeps
        # PE dense and spreads the DVE/gpsimd chain across the whole oc.
        class Epi:
            def __init__(self, oc, ctx_ps, s_ps):
                self.oc = oc
                self.ctx_ps = ctx_ps
                self.s_ps = s_ps
                self.xln = []

            def head(self):
                # free the S row bank + ctx_ps banks ASAP; split the copies
                # across ACT and DVE so neither queue's backlog delays the
                # next oc's first ctx matmul
                self.s_sb = rowp.tile([1, 512], F32, tag="s_sb")
                nc.scalar.copy(out=self.s_sb, in_=self.s_ps)
                self.ctx8 = ctxp.tile([P, EC, 512], FP8, tag="ctx8")
                for ec in range(EC):
                    eng_copy = (
                        nc.vector.tensor_copy if ec < 2 else nc.scalar.copy
                    )
                    eng_copy(out=self.ctx8[:, ec, :], in_=self.ctx_ps[:, ec, :])

            def scol(self):
                # S per-partition via tiny PE transposes (no reciprocal:
                # LayerNorm is scale-invariant per row, so we feed it
                # P = ctx_unnorm@Wc + S*x instead of x + (ctx_unnorm/S)@Wc)
                s_col = sps.tile([P, 4], F32, tag="sps")
                for j in range(4):
                    nc.tensor.matmul(
                        out=s_col[:, j : j + 1],
                        lhsT=self.s_sb[0:1, j * P : (j + 1) * P],
                        rhs=one32,
                        is_transpose=True,
                        start=(j == 0),
                        stop=(j == 3),
                        skip_group_check=True,
                    )
                self.s_colv = rowp.tile([P, 4], F32, tag="s_colv")
                nc.vector.tensor_copy(out=self.s_colv, in_=s_col)
                self.xsum4 = epi.tile([P, 4], F32, tag="xsum4")
                self.sq4 = epi.tile([P, 4], F32, tag="sq4")

            def wc(self, t):
                opre = sps.tile([P, 512], F32, tag="sps")
                for dc in (0, 2):
                    nc.tensor.matmul(
                        out=opre,
                        lhsT=self.ctx8[:, dc : dc + 2, t * P : (t + 1) * P],
                        rhs=wc8[:, dc : dc + 2, :],
                        start=(dc == 0),
                        stop=(dc == 2),
                        perf_mode=mybir.MatmulPerfMode.DoubleRow,
                        skip_group_check=True,
                    )
                # x_ln = S*xs + ctx@Wc, with sum(x_ln) accumulated for free;
                # the second moment runs on gpsimd so the DVE never does
                # bn_stats (keeps the DVE queue short at oc boundaries)
                x_ln = xlnp.tile([P, D], F32, tag="x_ln")
                nc.vector.scalar_tensor_tensor(
                    out=x_ln,
                    in0=xs_tiles[self.oc][:, t, :],
                    scalar=self.s_colv[:, t : t + 1],
                    in1=opre,
                    op0=mybir.AluOpType.mult,
                    op1=mybir.AluOpType.add,
                    accum_out=self.xsum4[:, t : t + 1],
                )
                self.xln.append(x_ln)
                junk = epi.tile([P, D], BF16, tag="junk")
                nc.scalar.activation(
                    out=junk,
                    in_=x_ln,
                    func=mybir.ActivationFunctionType.Square,
                    accum_out=self.sq4[:, t : t + 1],
                )

            def norm(self):
                # mean/var from raw moments (no bn_stats), then rsqrt as
                # exp(-0.5*ln(var+eps)): Ln and Exp live in the same ACT
                # table set, so the exp stream never swaps tables (Sqrt
                # lives in a different set and would thrash)
                self.m4 = epi.tile([P, 4], F32, tag="m4")
                nc.vector.tensor_scalar(
                    out=self.m4, in0=self.xsum4, scalar1=1.0 / D, scalar2=None,
                    op0=mybir.AluOpType.mult,
                )
                mm4 = epi.tile([P, 4], F32, tag="mm4")
                nc.vector.tensor_mul(mm4, self.m4, self.m4)
                v4 = epi.tile([P, 4], F32, tag="v4")
                nc.vector.scalar_tensor_tensor(
                    out=v4,
                    in0=self.sq4,
                    scalar=1.0 / D,
                    in1=mm4,
                    op0=mybir.AluOpType.mult,
                    op1=mybir.AluOpType.subtract,
                )
                lnv = epi.tile([P, 4], F32, tag="lnv")
                nc.scalar.activation(
                    out=lnv,
                    in_=v4,
                    func=mybir.ActivationFunctionType.Ln,
                    bias=eps_t[:, 0:1],
                    scale=1.0,
                )
                self.rs4 = epi.tile([P, 4], F32, tag="rs4")
                nc.scalar.activation(
                    out=self.rs4,
                    in_=lnv,
                    func=mybir.ActivationFunctionType.Exp,
                    bias=0.0,
                    scale=-0.5,
                )

            def pass2(self, t):
                t_sb = epi.tile([P, D], BF16, tag="t_sb")
                nc.vector.tensor_scalar(
                    t_sb,
                    self.xln[t],
                    self.m4[:, t : t + 1],
                    self.rs4[:, t : t + 1],
                    mybir.AluOpType.subtract,
                    mybir.AluOpType.mult,
                )
                g_sb = epi.tile([P, D], BF16, tag="g_sb")
                nc.gpsimd.tensor_mul(g_sb, t_sb, gamma_b16)
                o_sb = epi.tile([P, D], BF16, tag="o_sb")
                nc.vector.tensor_add(o_sb, g_sb, beta_b16)
                r0 = (self.oc * 4 + t) * P
                nc.sync.dma_start(out=out_ext[r0 : r0 + P, :], in_=o_sb)

        class OcState:
            def __init__(self, oc):
                self.oc = oc
                self.ctx_ps = ctxps_pool.tile([P, EC, 512], F32, tag="ctxps")
                self.s_ps = spsum.tile([1, 512], F32, tag="spsum")
                self.pt2 = None

        def emit_ib(st, ib):
            oc = st.oc
            if ib == 0 and oc + 1 < OC:
                for q in range(4):
                    load_e((oc + 1) * 4 + q)
            e8, il = e8_all[oc * 4 + ib // 4], ib % 4
            sp = sps.tile([P, 512], F32, tag="sps")
            for s in range(4):
                # start=True clears the whole PSUM bank -> only on s==0;
                # later mask MMs hit has_written=0 and write directly.
                nc.tensor.matmul(
                    out=sp[:, s * P : (s + 1) * P],
                    lhsT=e8[:, s, il * P : (il + 1) * P],
                    rhs=maskid8,
                    start=(s == 0),
                    stop=False,
                    skip_group_check=True,
                )
            for dc in (0, 2):
                nc.tensor.matmul(
                    out=sp,
                    lhsT=kt8[:, dc : dc + 2, ib * P : (ib + 1) * P],
                    rhs=qt8[:, dc : dc + 2, oc * 512 : (oc + 1) * 512],
                    start=False,
                    stop=(dc == 2),
                    perf_mode=mybir.MatmulPerfMode.DoubleRow,
                    skip_group_check=True,
                )
            if ib % 2 == 0:
                st.pt2 = ptp.tile([P, 2, 512], FP8, tag="pt")
            nc.scalar.activation(
                out=st.pt2[:, ib % 2, :],
                in_=sp,
                func=mybir.ActivationFunctionType.Exp,
                bias=cbias_t[:, 0:1],
                scale=ALPHA,
            )
            if ib % 2 == 1:
                j = (ib % 4) - 1
                for ec in range(EC):
                    nc.tensor.matmul(
                        out=st.ctx_ps[:, ec, :],
                        lhsT=x8g[ib // 4][:, j : j + 2, ec * P : (ec + 1) * P],
                        rhs=st.pt2,
                        start=(ib == 1),
                        stop=(ib == NB - 1),
                        perf_mode=mybir.MatmulPerfMode.DoubleRow,
                        skip_group_check=True,
                    )
                nc.tensor.matmul(
                    out=st.s_ps,
                    lhsT=ones8[:, :, 0:1],
                    rhs=st.pt2,
                    start=(ib == 1),
                    stop=(ib == NB - 1),
                    perf_mode=mybir.MatmulPerfMode.DoubleRow,
                    skip_group_check=True,
                )

        # prep runs ahead of the main loop (interleaving oc0's i-blocks
        # into prep was tried twice and loses: the x DMA stream physically
        # can't land before ~25us, and any interleaved score block waiting
        # on an edge quarter head-of-line blocks the in-order PE queue)
        for g in range(4):
            prep_group(g)

        st = OcState(0)
        for ib in range(NB):
            emit_ib(st, ib)

        pending = Epi(0, st.ctx_ps, st.s_ps)
        for oc in range(1, OC):
            st = OcState(oc)
            for ib in range(NB):
                emit_ib(st, ib)
                if ib == 0:
                    pending.head()
                elif ib == 1:
                    pending.scol()
                elif ib in (2, 4, 6, 8):
                    pending.wc(ib // 2 - 1)
                elif ib == 9:
                    pending.norm()
                elif ib in (10, 11, 12, 13):
                    # finish the DVE chain well before the oc ends so
                    # the next boundary's ctx8 casts start instantly
                    pending.pass2(ib - 10)
            pending = Epi(oc, st.ctx_ps, st.s_ps)

        # drain the final o-chunk's epilogue
        pending.head()
        pending.scol()
        for t in range(4):
            pending.wc(t)
        pending.norm()
        for t in range(4):
            pending.pass2(t)

    _split_excess_waits(nc)
    return nc


_NC_CACHE = None


def kernel(**inputs) -> np.ndarray:
    global _NC_CACHE
    _apply_patches()
    from concourse.bass_utils import run_bass_kernel_spmd

    node_fts = np.ascontiguousarray(np.asarray(inputs["node_fts"], dtype=np.float32))
    rel_edges = np.ascontiguousarray(np.asarray(inputs["rel_edges"], dtype=np.float32))
    shared = {
        k: np.ascontiguousarray(np.asarray(inputs[k], dtype=np.float32))
        for k in ("Wq", "bq", "Wk", "bk", "Wc", "gamma", "beta")
    }
    if _NC_CACHE is None:
        _NC_CACHE = build_nc()
    in_maps = [
        {"node_fts": node_fts[b], "rel_edges": rel_edges[b], **shared}
        for b in range(B)
    ]
    res = run_bass_kernel_spmd(_NC_CACHE, in_maps, core_ids=list(range(B)))
    return np.stack([res.results[b]["out"] for b in range(B)]).astype(np.float32)


# revision 42
# speedup vs baseline: 1.0911x; 1.0039x over previous
"""AttnGCN layer on 8 TRN2 NeuronCores — data-parallel over batch.

Per-core (one sample b):
  q = x @ Wq + bq ; k = x @ Wk + bk            (fp8 PE matmuls)
  sT[i,o] = k_i . q_o  + C'*eT[i,o]            (scores transposed; mask folded
                                                into PSUM via lhsT=e-block
                                                matmuls against a scaled
                                                identity — transposes e free)
  pT = exp(alpha*sT - C)                        (ACT, masked entries -> ~0)
  S[o] = sum_i pT[i,o]                          (ones-vector PE matmul)
  ctxT[e,o] = sum_i x8[i,e] * pT[i,o]           (PE, accumulated over i-blocks)
  P[o,c] = ctxT@Wc + S*x  ... LN(P) == LN(x + softmax@x@Wc) exactly
                                                (LayerNorm is scale-invariant
                                                 per row -> 1/S never needed)
  out = LN(P)*gamma + beta                      (DVE bn_stats/bn_aggr epilogue)

Self-contained: hardcodes shapes from the problem spec.
"""

import math
from contextlib import ExitStack

import numpy as np

import concourse.bass as bass
import concourse.tile as tile
from concourse import mybir
from concourse.vector_clock import ScopedClock

F32 = mybir.dt.float32
F32R = mybir.dt.float32r
BF16 = mybir.dt.bfloat16
FP8 = mybir.dt.float8e4

B = 8
N = 2048
D = 512
P = 128
NB = N // P       # 16 i-blocks
EC = D // P       # 4 chunks of the embed/dff dim
OC = N // 512     # 4 o-chunks of 512 attn rows
ALPHA = 1.0 / math.sqrt(D)
CPRIME = 128.0            # mask scale inside PSUM (fp8e4 max finite is 240)
SHIFT = 3.0               # softmax-invariant shift keeping exp() in fp8 range
CBIAS = CPRIME * ALPHA + SHIFT  # subtracted in the exp bias


# ---------------------------------------------------------------------------
# Workaround: walrus CoreV3 rejects >2 sem waits on the TileContext final
# drain ("Too many sync wait commands"). Hoist waits onto preceding nops.
def _patched_drain_and_barrier(self, tick_clock, wait_clock):
    nc = self.nc
    carrier = nc.sync.nop(nofuse=True)
    wait_clock.add_sem_waits(carrier.ins, ScopedClock({None: tick_clock.global_clock}))
    si = carrier.ins.sync_info
    waits = list(si.on_wait) if si and si.on_wait else []
    if len(waits) > 1:
        si.on_wait = waits[:1]
        for w in waits[1:]:
            n2 = nc.sync.nop(nofuse=True)
            n2.ins.sync_info = mybir.SyncInfo(on_wait=[w], on_update=[])
    nc.sync.drain()
    nc.all_engine_barrier()
    assert self.sems is not None
    popped = nc._tile_sem_poison_stack.pop()
    assert popped is self._sem_poison
    nc.clear_and_free_semaphores(list(self.sems.allocated().values()))
    nc.all_engine_barrier()


def _apply_patches():
    tile.TileContext._drain_and_barrier = _patched_drain_and_barrier


def _split_excess_waits(nc, limit=1):
    """walrus CoreV2/V3 codegen rejects instructions with >2 sem waits;
    hoist excess waits onto same-engine no-ops inserted just before."""
    n = 0
    for fn in nc.m.functions:
        for blk in fn.blocks:
            out = []
            changed = False
            for inst in blk.instructions:
                si = inst.sync_info
                waits = list(si.on_wait) if si and si.on_wait else []
                if len(waits) > limit:
                    keep = waits[-limit:]
                    for w in waits[:-limit]:
                        n += 1
                        nop = mybir.InstNoOp(name=f"I-wsplit-{n}", ins=[], outs=[])
                        nop.engine = inst.engine
                        nop.sync_info = mybir.SyncInfo(on_wait=[w], on_update=[])
                        out.append(nop)
                    si.on_wait = keep
                    changed = True
                out.append(inst)
            if changed:
                blk.instructions = out
    return n


def _identity(nc, ap, diag):
    nc.gpsimd.memset(ap, 0.0)
    nc.gpsimd.affine_select(
        out=ap,
        in_=ap,
        compare_op=mybir.AluOpType.not_equal,
        fill=diag,
        base=0,
        pattern=[[-1, ap.shape[0]]],
        channel_multiplier=1,
    )


def build_nc():
    nc = bass.Bass()
    x_ext = nc.declare_dram_parameter("node_fts", [N, D], F32, isOutput=False)
    e_ext = nc.declare_dram_parameter("rel_edges", [N, N], F32, isOutput=False)
    wq_ext = nc.declare_dram_parameter("Wq", [D, D], F32, isOutput=False)
    bq_ext = nc.declare_dram_parameter("bq", [D], F32, isOutput=False)
    wk_ext = nc.declare_dram_parameter("Wk", [D, D], F32, isOutput=False)
    bk_ext = nc.declare_dram_parameter("bk", [D], F32, isOutput=False)
    wc_ext = nc.declare_dram_parameter("Wc", [D, D], F32, isOutput=False)
    g_ext = nc.declare_dram_parameter("gamma", [D], F32, isOutput=False)
    be_ext = nc.declare_dram_parameter("beta", [D], F32, isOutput=False)
    out_ext = nc.declare_dram_parameter("out", [N, D], BF16, isOutput=True)

    with tile.TileContext(nc) as tc, ExitStack() as ctx:
        singles = ctx.enter_context(tc.tile_pool(name="singles", bufs=1))
        wstage = ctx.enter_context(tc.tile_pool(name="wstage", bufs=2))
        xtp = ctx.enter_context(tc.tile_pool(name="xtp", bufs=2))
        ptp = ctx.enter_context(tc.tile_pool(name="ptp", bufs=4))
        ctxp = ctx.enter_context(tc.tile_pool(name="ctxp", bufs=2))
        rowp = ctx.enter_context(tc.tile_pool(name="rowp", bufs=2))
        epi = ctx.enter_context(tc.tile_pool(name="epi", bufs=2))
        xlnp = ctx.enter_context(tc.tile_pool(name="xlnp", bufs=5))
        sps = ctx.enter_context(tc.tile_pool(name="sps", bufs=3, space="PSUM"))
        ctxps_pool = ctx.enter_context(tc.tile_pool(name="ctxps", bufs=1, space="PSUM"))
        spsum = ctx.enter_context(tc.tile_pool(name="spsum", bufs=1, space="PSUM"))

        # ---- persistent tiles -------------------------------------------
        xs_tiles = [
            singles.tile([P, 4, D], F32, tag=f"xs{g}", name=f"xs{g}")
            for g in range(4)
        ]
        x8g = [
            singles.tile([P, 4, D], FP8, tag=f"x8g{g}", name=f"x8g{g}")
            for g in range(4)
        ]
        # 16 e-quarters, fp8, each [128 (o in block), 4 (o blocks), 512 (i)]
        e8_all = [
            singles.tile([P, 4, 512], FP8, tag=f"e8_{j}", name=f"e8_{j}")
            for j in range(16)
        ]
        qt8 = singles.tile([P, EC, N], FP8, tag="qt8")
        kt8 = singles.tile([P, EC, N], FP8, tag="kt8")
        wq8 = singles.tile([P, EC, D], FP8, tag="wq8")
        wk8 = singles.tile([P, EC, D], FP8, tag="wk8")
        wc8 = singles.tile([P, EC, D], FP8, tag="wc8")
        bq_row = singles.tile([1, D], F32, tag="bq_row")
        bk_row = singles.tile([1, D], F32, tag="bk_row")
        g_row = singles.tile([1, D], F32, tag="g_row")
        be_row = singles.tile([1, D], F32, tag="be_row")
        bqkt = singles.tile([P, 2 * EC], F32, tag="bqkt")
        gamma_b16 = singles.tile([P, D], BF16, tag="gamma_b16")
        beta_b16 = singles.tile([P, D], BF16, tag="beta_b16")
        ident32 = singles.tile([P, P], F32, tag="ident32")
        maskid8 = singles.tile([P, P], FP8, tag="maskid8")
        ones8 = singles.tile([P, 2, 16], FP8, tag="ones8")
        one32 = singles.tile([1, 1], F32, tag="one32")
        eps_t = singles.tile([P, 1], F32, tag="eps_t")
        cbias_t = singles.tile([P, 1], F32, tag="cbias_t")

        # gpsimd queue: identity build only, then DMA triggers ASAP;
        # the rest of the constants go on DVE so the SWDGE loads start early
        _identity(nc, ident32, 1.0)
        nc.vector.tensor_scalar(
            out=maskid8, in0=ident32, scalar1=CPRIME, scalar2=None,
            op0=mybir.AluOpType.mult,
        )
        nc.vector.memset(ones8, 1.0)
        nc.vector.memset(one32, 1.0)
        nc.vector.memset(eps_t, 1e-5)
        nc.vector.memset(cbias_t, -CBIAS)

        # ---- input loads, split across the three DMA paths --------------
        # sync+scalar HWDGE: x (f32, prep-critical; the ~1us trigger cost
        # per DMA serializes on one engine); SWDGE (casting): weights->fp8
        # then edges->fp8. Non-critical loads get tile_wait_until so the
        # priority scheduler doesn't float them into the x stream's window
        # (dep-free DMA triggers otherwise all issue at t~0 and the SDMA
        # round-robin starves x: xs3 was landing at 29-33us).
        def load_xs(g):
            eng = nc.sync if g % 2 == 0 else nc.scalar
            eng.dma_start(
                out=xs_tiles[g],
                in_=x_ext[g * 4 * P : (g + 1) * 4 * P, :].rearrange(
                    "(ib p) e -> p ib e", p=P
                ),
            )

        def load_e(j):  # j = oc*4 + q
            oc, q = j // 4, j % 4
            nc.gpsimd.dma_start(
                out=e8_all[j],
                in_=e_ext[
                    oc * 512 : (oc + 1) * 512, q * 512 : (q + 1) * 512
                ].rearrange("(s p) f -> p s f", p=P),
            )

        def load_w(w_ext, w8):
            nc.gpsimd.dma_start(
                out=w8, in_=w_ext[:, :].rearrange("(ec p) f -> p ec f", p=P)
            )

        # prep-critical loads only; e-quarters for oc>0 are issued inside the
        # main loop (one oc ahead) so they don't steal DMA bandwidth from x/w
        for g in range(4):
            load_xs(g)
        load_w(wq_ext, wq8)
        load_w(wk_ext, wk8)
        for j, ms in enumerate((0.010, 0.012, 0.014, 0.016)):
            with tc.tile_wait_until(ms):
                load_e(j)
        with tc.tile_wait_until(0.018):
            load_w(wc_ext, wc8)

        # ---- HAM warmup: dummy matmul burst while the first DMAs land ----
        # PE clock-gate needs ~3.4us of sustained activity to go 1.2->2.4GHz;
        # burn the DMA-wait with throwaway matmuls so prep runs warm. The
        # operand is an uninitialized tile (no writer -> no deps) so the
        # burst starts right after the NEFF preamble, before the constants.
        warm_src = singles.tile([P, P], FP8, tag="warm_src")
        nc.vector.memset(warm_src, 1.0)
        warm_ps = sps.tile([P, 512], F32, tag="sps")
        for j in range(48):
            nc.tensor.matmul(
                out=warm_ps[:, (j % 4) * P : (j % 4 + 1) * P],
                lhsT=warm_src,
                rhs=warm_src,
                start=True,
                stop=True,
                skip_group_check=True,
            )

        # vector params as single-row loads (1 descriptor each; the old
        # [128,D] broadcast DMAs generated 4-byte descriptors and cost
        # 8.3us of engine time per trigger). Broadcast/transpose on-chip.
        def _row(ap_1d):
            return bass.AP(
                tensor=ap_1d.tensor, offset=ap_1d.offset, ap=[[0, 1], *ap_1d.ap]
            )

        with tc.tile_wait_until(0.006):
            nc.scalar.dma_start(out=bq_row, in_=_row(bq_ext[:]))
            nc.scalar.dma_start(out=bk_row, in_=_row(bk_ext[:]))
            nc.scalar.dma_start(out=g_row, in_=_row(g_ext[:]))
            nc.scalar.dma_start(out=be_row, in_=_row(be_ext[:]))

        # broadcast rows across partitions via a K=1 PE matmul
        # (ones_col.T @ row), then one cast each to bf16
        ones_col = singles.tile([1, P], F32, tag="ones_col")
        nc.vector.memset(ones_col, 1.0)
        for row, dst in ((g_row, gamma_b16), (be_row, beta_b16)):
            bc_ps = sps.tile([P, 512], F32, tag="sps")
            nc.tensor.matmul(
                out=bc_ps,
                lhsT=ones_col,
                rhs=row,
                start=True,
                stop=True,
                skip_group_check=True,
            )
            nc.vector.tensor_copy(out=dst, in_=bc_ps)

        # biases per-partition [p, fc] (f = fc*128 + p) via tiny PE
        # transposes, like the S column later
        bqk_ps = sps.tile([P, 2 * EC], F32, tag="sps")
        for j in range(8):
            src = bq_row if j < 4 else bk_row
            nc.tensor.matmul(
                out=bqk_ps[:, j : j + 1],
                lhsT=src[0:1, (j % 4) * P : (j % 4 + 1) * P],
                rhs=one32,
                is_transpose=True,
                start=(j == 0),
                stop=(j == 7),
                skip_group_check=True,
            )
        nc.vector.tensor_copy(out=bqkt, in_=bqk_ps)
        bqt = bqkt[:, 0:EC]
        bkt = bqkt[:, EC : 2 * EC]

        # ---- per-group prep: x8 cast, transposes, q/k projections -------
        def prep_group(g):
            xs = xs_tiles[g]
            nc.vector.tensor_copy(out=x8g[g], in_=xs)
            xt = xtp.tile([P, EC, 512], FP8, tag="xt")
            for ec in range(EC):
                tp = sps.tile([P, 512], F32, tag="sps")
                for k4 in range(4):
                    nc.tensor.transpose(
                        out=tp[:, k4 * P : (k4 + 1) * P],
                        in_=xs[:, k4, ec * P : (ec + 1) * P],
                        identity=ident32,
                    )
                nc.vector.tensor_copy(out=xt[:, ec, :], in_=tp)
            for w8, bt, dst in ((wq8, bqt, qt8), (wk8, bkt, kt8)):
                for fc in range(EC):
                    ps = sps.tile([P, 512], F32, tag="sps")
                    for dc in (0, 2):
                        nc.tensor.matmul(
                            out=ps,
                            lhsT=w8[:, dc : dc + 2, fc * P : (fc + 1) * P],
                            rhs=xt[:, dc : dc + 2, :],
                            start=(dc == 0),
                            stop=(dc == 2),
                            perf_mode=mybir.MatmulPerfMode.DoubleRow,
                            skip_group_check=True,
                        )
                    # bias + fp8 cast: q on ACT, k on DVE — splitting the
                    # PSUM->SBUF drain keeps either engine off the prep
                    # critical path (ACT alone was the prep bottleneck)
                    if dst is qt8:
                        nc.scalar.activation(
                            out=dst[:, fc, g * 512 : (g + 1) * 512],
                            in_=ps,
                            func=mybir.ActivationFunctionType.Identity,
                            bias=bt[:, fc : fc + 1],
                            scale=1.0,
                        )
                    else:
                        nc.vector.tensor_scalar(
                            out=dst[:, fc, g * 512 : (g + 1) * 512],
                            in0=ps,
                            scalar1=bt[:, fc : fc + 1],
                            scalar2=None,
                            op0=mybir.AluOpType.add,
                        )

        # ---- main loop over o-chunks ------------------------------------
        # The per-oc epilogue is software-pipelined into the NEXT oc's
        # i-block loop: the PE engine queue is in-order, so emitting
        # epilogue PE work (s_col transposes, Wc matmuls) contiguously at
        # the oc boundary head-of-line blocks ready score matmuls behind a
        # DVE dependency chain. Interleaving stages at fixed i-blocks keeps
        # PE dense and spreads the DVE/gpsimd chain across the whole oc.
        class Epi:
            def __init__(self, oc, ctx_ps, s_ps):
                self.oc = oc
                self.ctx_ps = ctx_ps
                self.s_ps = s_ps
                self.xln = []

            def head(self):
                # free the S row bank + ctx_ps banks ASAP; split the copies
                # across ACT and DVE so neither queue's backlog delays the
                # next oc's first ctx matmul
                self.s_sb = rowp.tile([1, 512], F32, tag="s_sb")
                nc.scalar.copy(out=self.s_sb, in_=self.s_ps)
                self.ctx8 = ctxp.tile([P, EC, 512], FP8, tag="ctx8")
                for ec in range(EC):
                    # 3 on DVE, 1 on ACT: the ACT queue feeds the exp
                    # stream that paces the PE, so keep its boundary
                    # backlog to one copy
                    eng_copy = (
                        nc.vector.tensor_copy if ec < 3 else nc.scalar.copy
                    )
                    eng_copy(out=self.ctx8[:, ec, :], in_=self.ctx_ps[:, ec, :])

            def scol(self):
                # S per-partition via tiny PE transposes (no reciprocal:
                # LayerNorm is scale-invariant per row, so we feed it
                # P = ctx_unnorm@Wc + S*x instead of x + (ctx_unnorm/S)@Wc)
                s_col = sps.tile([P, 4], F32, tag="sps")
                for j in range(4):
                    nc.tensor.matmul(
                        out=s_col[:, j : j + 1],
                        lhsT=self.s_sb[0:1, j * P : (j + 1) * P],
                        rhs=one32,
                        is_transpose=True,
                        start=(j == 0),
                        stop=(j == 3),
                        skip_group_check=True,
                    )
                self.s_colv = rowp.tile([P, 4], F32, tag="s_colv")
                nc.vector.tensor_copy(out=self.s_colv, in_=s_col)
                self.xsum4 = epi.tile([P, 4], F32, tag="xsum4")
                self.sq4 = epi.tile([P, 4], F32, tag="sq4")

            def wc(self, t):
                opre = sps.tile([P, 512], F32, tag="sps")
                for dc in (0, 2):
                    nc.tensor.matmul(
                        out=opre,
                        lhsT=self.ctx8[:, dc : dc + 2, t * P : (t + 1) * P],
                        rhs=wc8[:, dc : dc + 2, :],
                        start=(dc == 0),
                        stop=(dc == 2),
                        perf_mode=mybir.MatmulPerfMode.DoubleRow,
                        skip_group_check=True,
                    )
                # x_ln = S*xs + ctx@Wc, with sum(x_ln) accumulated for free;
                # the second moment runs on gpsimd so the DVE never does
                # bn_stats (keeps the DVE queue short at oc boundaries)
                x_ln = xlnp.tile([P, D], F32, tag="x_ln")
                nc.vector.scalar_tensor_tensor(
                    out=x_ln,
                    in0=xs_tiles[self.oc][:, t, :],
                    scalar=self.s_colv[:, t : t + 1],
                    in1=opre,
                    op0=mybir.AluOpType.mult,
                    op1=mybir.AluOpType.add,
                    accum_out=self.xsum4[:, t : t + 1],
                )
                self.xln.append(x_ln)
                junk = epi.tile([P, D], BF16, tag="junk")
                nc.scalar.activation(
                    out=junk,
                    in_=x_ln,
                    func=mybir.ActivationFunctionType.Square,
                    accum_out=self.sq4[:, t : t + 1],
                )

            def norm(self):
                # mean/var from raw moments (no bn_stats), then rsqrt as
                # exp(-0.5*ln(var+eps)): Ln and Exp live in the same ACT
                # table set, so the exp stream never swaps tables (Sqrt
                # lives in a different set and would thrash)
                self.m4 = epi.tile([P, 4], F32, tag="m4")
                nc.vector.tensor_scalar(
                    out=self.m4, in0=self.xsum4, scalar1=1.0 / D, scalar2=None,
                    op0=mybir.AluOpType.mult,
                )
                mm4 = epi.tile([P, 4], F32, tag="mm4")
                nc.vector.tensor_mul(mm4, self.m4, self.m4)
                v4 = epi.tile([P, 4], F32, tag="v4")
                nc.vector.scalar_tensor_tensor(
                    out=v4,
                    in0=self.sq4,
                    scalar=1.0 / D,
                    in1=mm4,
                    op0=mybir.AluOpType.mult,
                    op1=mybir.AluOpType.subtract,
                )
                lnv = epi.tile([P, 4], F32, tag="lnv")
                nc.scalar.activation(
                    out=lnv,
                    in_=v4,
                    func=mybir.ActivationFunctionType.Ln,
                    bias=eps_t[:, 0:1],
                    scale=1.0,
                )
                self.rs4 = epi.tile([P, 4], F32, tag="rs4")
                nc.scalar.activation(
                    out=self.rs4,
                    in_=lnv,
                    func=mybir.ActivationFunctionType.Exp,
                    bias=0.0,
                    scale=-0.5,
                )

            def pass2(self, t):
                t_sb = epi.tile([P, D], BF16, tag="t_sb")
                nc.vector.tensor_scalar(
                    t_sb,
                    self.xln[t],
                    self.m4[:, t : t + 1],
                    self.rs4[:, t : t + 1],
                    mybir.AluOpType.subtract,
                    mybir.AluOpType.mult,
                )
                # alternate gamma/beta engines by tile so neither gpsimd
                # nor DVE serializes all four chains at the kernel tail
                g_sb = epi.tile([P, D], BF16, tag="g_sb")
                o_sb = epi.tile([P, D], BF16, tag="o_sb")
                if t % 2 == 0:
                    nc.gpsimd.tensor_mul(g_sb, t_sb, gamma_b16)
                    nc.vector.tensor_add(o_sb, g_sb, beta_b16)
                else:
                    nc.vector.tensor_mul(g_sb, t_sb, gamma_b16)
                    nc.gpsimd.tensor_add(o_sb, g_sb, beta_b16)
                r0 = (self.oc * 4 + t) * P
                nc.sync.dma_start(out=out_ext[r0 : r0 + P, :], in_=o_sb)

        class OcState:
            def __init__(self, oc):
                self.oc = oc
                self.ctx_ps = ctxps_pool.tile([P, EC, 512], F32, tag="ctxps")
                self.s_ps = spsum.tile([1, 512], F32, tag="spsum")
                self.pt2 = None

        def emit_ib(st, ib):
            oc = st.oc
            if ib == 0 and oc + 1 < OC:
                for q in range(4):
                    load_e((oc + 1) * 4 + q)
            e8, il = e8_all[oc * 4 + ib // 4], ib % 4
            sp = sps.tile([P, 512], F32, tag="sps")
            for s in range(4):
                # start=True clears the whole PSUM bank -> only on s==0;
                # later mask MMs hit has_written=0 and write directly.
                nc.tensor.matmul(
                    out=sp[:, s * P : (s + 1) * P],
                    lhsT=e8[:, s, il * P : (il + 1) * P],
                    rhs=maskid8,
                    start=(s == 0),
                    stop=False,
                    skip_group_check=True,
                )
            for dc in (0, 2):
                nc.tensor.matmul(
                    out=sp,
                    lhsT=kt8[:, dc : dc + 2, ib * P : (ib + 1) * P],
                    rhs=qt8[:, dc : dc + 2, oc * 512 : (oc + 1) * 512],
                    start=False,
                    stop=(dc == 2),
                    perf_mode=mybir.MatmulPerfMode.DoubleRow,
                    skip_group_check=True,
                )
            if ib % 2 == 0:
                st.pt2 = ptp.tile([P, 2, 512], FP8, tag="pt")
            nc.scalar.activation(
                out=st.pt2[:, ib % 2, :],
                in_=sp,
                func=mybir.ActivationFunctionType.Exp,
                bias=cbias_t[:, 0:1],
                scale=ALPHA,
            )
            if ib % 2 == 1:
                j = (ib % 4) - 1
                for ec in range(EC):
                    nc.tensor.matmul(
                        out=st.ctx_ps[:, ec, :],
                        lhsT=x8g[ib // 4][:, j : j + 2, ec * P : (ec + 1) * P],
                        rhs=st.pt2,
                        start=(ib == 1),
                        stop=(ib == NB - 1),
                        perf_mode=mybir.MatmulPerfMode.DoubleRow,
                        skip_group_check=True,
                    )
                nc.tensor.matmul(
                    out=st.s_ps,
                    lhsT=ones8[:, :, 0:1],
                    rhs=st.pt2,
                    start=(ib == 1),
                    stop=(ib == NB - 1),
                    perf_mode=mybir.MatmulPerfMode.DoubleRow,
                    skip_group_check=True,
                )

        # prep runs ahead of the main loop (interleaving oc0's i-blocks
        # into prep was tried twice and loses: the x DMA stream physically
        # can't land before ~25us, and any interleaved score block waiting
        # on an edge quarter head-of-line blocks the in-order PE queue)
        for g in range(4):
            prep_group(g)

        st = OcState(0)
        for ib in range(NB):
            emit_ib(st, ib)

        pending = Epi(0, st.ctx_ps, st.s_ps)
        for oc in range(1, OC):
            st = OcState(oc)
            for ib in range(NB):
                emit_ib(st, ib)
                if ib == 0:
                    pending.head()
                elif ib == 1:
                    pending.scol()
                elif ib in (2, 4, 6, 8):
                    pending.wc(ib // 2 - 1)
                elif ib == 9:
                    pending.norm()
                elif ib in (10, 11, 12, 13):
                    # finish the DVE chain well before the oc ends so
                    # the next boundary's ctx8 casts start instantly
                    pending.pass2(ib - 10)
            pending = Epi(oc, st.ctx_ps, st.s_ps)

        # drain the final o-chunk's epilogue
        pending.head()
        pending.scol()
        for t in range(4):
            pending.wc(t)
        pending.norm()
        for t in range(4):
            pending.pass2(t)

    _split_excess_waits(nc)
    return nc


_NC_CACHE = None


def kernel(**inputs) -> np.ndarray:
    global _NC_CACHE
    _apply_patches()
    from concourse.bass_utils import run_bass_kernel_spmd

    node_fts = np.ascontiguousarray(np.asarray(inputs["node_fts"], dtype=np.float32))
    rel_edges = np.ascontiguousarray(np.asarray(inputs["rel_edges"], dtype=np.float32))
    shared = {
        k: np.ascontiguousarray(np.asarray(inputs[k], dtype=np.float32))
        for k in ("Wq", "bq", "Wk", "bk", "Wc", "gamma", "beta")
    }
    if _NC_CACHE is None:
        _NC_CACHE = build_nc()
    in_maps = [
        {"node_fts": node_fts[b], "rel_edges": rel_edges[b], **shared}
        for b in range(B)
    ]
    res = run_bass_kernel_spmd(_NC_CACHE, in_maps, core_ids=list(range(B)))
    return np.stack([res.results[b]["out"] for b in range(B)]).astype(np.float32)
